# revision 1
# baseline (speedup 1.0000x reference)
"""DelayGNN stage kernel for 8 Trainium2 NeuronCores.

Strategy (graph/data parallel):
  - Nodes sharded across 8 cores (6400 padded nodes each); edge lists
    partitioned by destination core, sorted by destination block, grouped
    into 256-node destination blocks, padded to uniform chunk counts so one
    SPMD program serves all cores.
  - Per layer: bulk-gather x[src] rows (512B) from a replicated DRAM table
    with dma_gather (int16 indices; the node table is split in two halves to
    fit the int16 range), scatter-add into per-block accumulators with
    one-hot matmuls on the tensor engine (float32r), dense W matmuls in true
    fp32, row-layout epilogue (relu + residual + L2 normalize), then an
    AllGather of the new node features; the hop-2 aggregation (only needed
    by the next layer) overlaps the AllGather.
"""
import os
import sys
import numpy as np

for _p in ("/opt/trn_rl_repo", "/root/.axon_site/_ro/trn_rl_repo"):
    if os.path.isdir(_p) and _p not in sys.path:
        sys.path.append(_p)

P = 128
BLK = 256
NCORES = 8
HALF = 32768  # int16 index ceiling per gather table half


def _wrap_idx(flat):
    """[n] int -> dma_gather idx layout [128, n/16] (wrapped, replicated)."""
    n = len(flat)
    w = np.asarray(flat, np.int16).reshape(n // 16, 16).T  # [16, n/16]
    return np.ascontiguousarray(np.tile(w, (8, 1)))


def _prep_hop(src, dst, norm, n_per_core, nblk):
    """Partition edges by dst core, sort by (dst block, src half), pad each
    block to CA + CB chunks of 128. Returns (CA, CB, per-core tables)."""
    core = dst // n_per_core
    percore = []
    cntA = np.zeros(nblk, np.int64)
    cntB = np.zeros(nblk, np.int64)
    for k in range(NCORES):
        sel = core == k
        s, d, w = src[sel], dst[sel] - k * n_per_core, norm[sel]
        blk = d // BLK
        isB = (s >= HALF).astype(np.int64)
        order = np.lexsort((isB, blk))
        s, d, w, blk, isB = (a[order] for a in (s, d, w, blk, isB))
        grp = blk * 2 + isB
        cnt = np.bincount(grp, minlength=2 * nblk)
        starts = np.concatenate([[0], np.cumsum(cnt)[:-1]])
        rank = np.arange(len(s)) - starts[grp]
        percore.append((s, d, w, blk, isB, rank))
        cntA = np.maximum(cntA, cnt[0::2])
        cntB = np.maximum(cntB, cnt[1::2])
    # per-block chunk counts (max over cores -> SPMD-uniform program)
    CAb = np.maximum(1, -(-cntA // P)).astype(np.int64)
    CBb = (-(-cntB // P)).astype(np.int64)
    Cgb = CAb + CBb
    choff = np.concatenate([[0], np.cumsum(Cgb)])       # chunk offsets
    aoff = np.concatenate([[0], np.cumsum(CAb)])        # A-chunk offsets
    boff = np.concatenate([[0], np.cumsum(CBb)])        # B-chunk offsets
    J = int(Cgb.sum())
    out = []
    for k in range(NCORES):
        s, d, w, blk, isB, rank = percore[k]
        gix = np.zeros(J * P, np.int64)
        dp = np.zeros(J * P, np.float32)
        wp = np.zeros(J * P, np.float32)
        pos = choff[blk] * P + isB * (CAb[blk] * P) + rank
        gix[pos] = np.where(isB == 1, s - HALF, s)
        dp[pos] = (d % BLK).astype(np.float32)
        wp[pos] = w
        idxA = np.concatenate(
            [_wrap_idx(gix[choff[b] * P:(choff[b] + CAb[b]) * P])
             for b in range(nblk)], axis=1)
        idxB = (np.concatenate(
            [_wrap_idx(gix[(choff[b] + CAb[b]) * P:choff[b + 1] * P])
             for b in range(nblk) if CBb[b]], axis=1)
            if CBb.sum() else None)
        out.append((
            np.ascontiguousarray(dp.reshape(-1, P).T),
            np.ascontiguousarray(wp.reshape(-1, P).T),
            idxA, idxB,
        ))
    return tuple(CAb), tuple(CBb), out


def _edge_norm(src, dst, n):
    ones = np.ones(len(src), np.float32)
    deg_out = np.bincount(src, weights=ones, minlength=n).astype(np.float32)
    deg_in = np.bincount(dst, weights=ones, minlength=n).astype(np.float32)
    inv_out = np.where(deg_out > 0,
                       (1.0 / np.sqrt(np.maximum(deg_out, 1.0))), 0.0)
    inv_in = np.where(deg_in > 0,
                      (1.0 / np.sqrt(np.maximum(deg_in, 1.0))), 0.0)
    return (inv_out[src] * inv_in[dst]).astype(np.float32)


def _softmax(v):
    e = np.exp(v - v.max())
    return (e / e.sum()).astype(np.float32)


def _build(nblk, CA1, CB1, CA2, CB2, L, has_bias, msg_dt_name="float32r"):
    """Build the SPMD Bass program. nblk 256-dst blocks per core."""
    import concourse.bass as bass
    import concourse.tile as tile
    from concourse import bacc, mybir
    from concourse.library_config import mlp
    from contextlib import ExitStack

    F32 = mybir.dt.float32
    I16 = mybir.dt.int16
    MSG = getattr(mybir.dt, msg_dt_name)
    NP = nblk * BLK            # nodes per core
    NPAD = NP * NCORES
    HB = min(HALF, NPAD)       # rows in table half A
    NBN = NP // P              # 128-node blocks per core
    CAs, CBs = (np.asarray(CA1), np.asarray(CA2)), (np.asarray(CB1),
                                                     np.asarray(CB2))
    choffs = [np.concatenate([[0], np.cumsum(CAs[h] + CBs[h])])
              for h in range(2)]
    aoffs = [np.concatenate([[0], np.cumsum(CAs[h])]) for h in range(2)]
    boffs = [np.concatenate([[0], np.cumsum(CBs[h])]) for h in range(2)]
    Js = (int(choffs[0][-1]), int(choffs[1][-1]))

    nc = bacc.Bacc("TRN2", target_bir_lowering=False, debug=False,
                   num_devices=NCORES)

    x_full = nc.dram_tensor("x_full", [NPAD, P], F32, kind="ExternalInput")
    x_own = nc.dram_tensor("x_own", [NP, P], F32, kind="ExternalInput")
    w1d = nc.dram_tensor("w1s", [L, P, P], F32, kind="ExternalInput")
    w2d = nc.dram_tensor("w2s", [L, P, P], F32, kind="ExternalInput")
    biasd = nc.dram_tensor("biasb", [L, P, P], F32, kind="ExternalInput")
    iotad = nc.dram_tensor("iota", [P, BLK], F32, kind="ExternalInput")
    dstd = [nc.dram_tensor(f"dst{h}", [P, J], F32, kind="ExternalInput")
            for h, J in ((1, Js[0]), (2, Js[1]))]
    nrmd = [nc.dram_tensor(f"nrm{h}", [P, J], F32, kind="ExternalInput")
            for h, J in ((1, Js[0]), (2, Js[1]))]
    idxad = [nc.dram_tensor(f"idxa{h+1}", [P, int(aoffs[h][-1]) * 8], I16,
                            kind="ExternalInput") for h in range(2)]
    idxbd = [nc.dram_tensor(f"idxb{h+1}", [P, int(boffs[h][-1]) * 8], I16,
                            kind="ExternalInput") if CBs[h].sum() else None
             for h in range(2)]
    out_own = nc.dram_tensor("out_own", [NP, P], F32, kind="ExternalOutput")

    ag_in = [nc.dram_tensor(f"ag_in{t}", [NP, P], F32, kind="Internal")
             for t in range(L - 1)]
    ag_out = [nc.dram_tensor(f"ag_out{t}", [NPAD, P], F32, kind="Internal",
                             addr_space="Shared")
              for t in range(L - 1)]

    with tile.TileContext(nc) as tc, ExitStack() as ctx:
        sb = ctx.enter_context(tc.tile_pool(name="sb", bufs=1))
        gpool = ctx.enter_context(tc.tile_pool(name="g", bufs=3))
        ohpool = ctx.enter_context(tc.tile_pool(name="oh", bufs=4))
        accp = ctx.enter_context(
            tc.tile_pool(name="accp", bufs=2, space="PSUM"))
        densep = ctx.enter_context(
            tc.tile_pool(name="densep", bufs=2, space="PSUM"))
        misc = ctx.enter_context(tc.tile_pool(name="misc", bufs=2))

        # --- persistent tiles ---
        t_dst = [sb.tile([P, Js[0]], F32, tag="dst1", name="tdst1"),
                 sb.tile([P, Js[1]], F32, tag="dst2", name="tdst2")]
        t_nrm = [sb.tile([P, Js[0]], F32, tag="nrm1", name="tnrm1"),
                 sb.tile([P, Js[1]], F32, tag="nrm2", name="tnrm2")]
        t_ixa = [sb.tile([P, int(aoffs[h][-1]) * 8], I16, tag=f"ixa{h}",
                         name=f"ixa{h}") for h in range(2)]
        t_ixb = [sb.tile([P, int(boffs[h][-1]) * 8], I16, tag=f"ixb{h}",
                         name=f"ixb{h}") if CBs[h].sum() else None
                 for h in range(2)]
        t_iota = sb.tile([P, BLK], F32, tag="iota")
        t_w1 = sb.tile([P, L, P], F32, tag="w1")
        t_w2 = sb.tile([P, L, P], F32, tag="w2")
        t_bias = sb.tile([P, L, P], F32, tag="bias")
        x_rows = [sb.tile([P, NBN, P], F32, tag=f"xr{i}", name=f"xr{i}")
                  for i in range(2)]
        acc1 = sb.tile([P, nblk, BLK], F32, tag="acc1")
        acc2 = [sb.tile([P, nblk, BLK], F32, tag=f"acc2_{i}",
                        name=f"acc2_{i}") for i in range(2)]
        ssum = sb.tile([P, NBN], F32, tag="ssum")
        sinv = sb.tile([P, NBN], F32, tag="sinv")

        nc.gpsimd.load_library(mlp)
        for h in range(2):
            nc.sync.dma_start(t_dst[h][:], dstd[h][:])
            nc.sync.dma_start(t_nrm[h][:], nrmd[h][:])
            nc.sync.dma_start(t_ixa[h][:], idxad[h][:])
            if CBs[h].sum():
                nc.sync.dma_start(t_ixb[h][:], idxbd[h][:])
        nc.sync.dma_start(t_iota[:], iotad[:])
        nc.sync.dma_start(t_w1[:], w1d[:].rearrange("t i o -> i t o"))
        nc.sync.dma_start(t_w2[:], w2d[:].rearrange("t i o -> i t o"))
        nc.sync.dma_start(t_bias[:], biasd[:].rearrange("t i o -> i t o"))
        nc.sync.dma_start(x_rows[0][:],
                          x_own[:].rearrange("(a p) f -> p a f", p=P))

        MAXC = 8  # dma_gather descriptor-ring cap: <=1024 idxs per call

        def gather_pieces(tile_tag, tab_ap, idx_tile, ch0, C):
            """Gather C chunks (idx-table chunk offset ch0) in pieces of
            <=MAXC chunks. Returns [(first_chunk, piece_ap)]."""
            pieces = []
            for p0 in range(0, C, MAXC):
                pc = min(MAXC, C - p0)
                ni = pc * P
                xg = gpool.tile([P, pc, P], MSG, tag=f"{tile_tag}{p0}",
                                name=f"{tile_tag}{p0}")
                col0 = (ch0 + p0) * 8
                nc.gpsimd.dma_gather(
                    out_ap=xg[:], in_ap=tab_ap,
                    idxs_ap=idx_tile[:, col0:col0 + pc * 8],
                    num_idxs=ni, num_idxs_reg=ni, elem_size=P)
                pieces.append((p0, xg))
            return pieces

        def scatter_hop(h, acc_sb, xsrc):
            tab = xsrc.bitcast(MSG)
            for b in range(nblk):
                CA, CB = int(CAs[h][b]), int(CBs[h][b])
                Cg = CA + CB
                pa = gather_pieces("xga", tab[0:HB, :], t_ixa[h],
                                   int(aoffs[h][b]), CA)
                pb = (gather_pieces("xgb", tab[HB:NPAD, :], t_ixb[h],
                                    int(boffs[h][b]), CB)
                      if CB else [])

                def chunk_ap(c):
                    pieces, cc = (pa, c) if c < CA else (pb, c - CA)
                    for p0, xg in reversed(pieces):
                        if cc >= p0:
                            return xg[:, cc - p0, :]
                    raise AssertionError

                ps = accp.tile([P, BLK], F32, tag="psacc", space="PSUM")
                for c in range(Cg):
                    j = int(choffs[h][b]) + c
                    xsl = chunk_ap(c)
                    oh = ohpool.tile([P, BLK], MSG, tag="oh")
                    nc.vector.tensor_scalar(
                        out=oh[:], in0=t_iota[:],
                        scalar1=t_dst[h][:, j:j + 1],
                        scalar2=t_nrm[h][:, j:j + 1],
                        op0=mybir.AluOpType.is_equal,
                        op1=mybir.AluOpType.mult,
                    )
                    nc.tensor.matmul(out=ps[:], lhsT=xsl, rhs=oh[:],
                                     start=(c == 0), stop=(c == Cg - 1))
                nc.scalar.copy(acc_sb[:, b, :], ps[:])

        for t in range(L):
            xsrc = x_full[:] if t == 0 else ag_out[t - 1][:]
            xcur = x_rows[t % 2]
            xnew = x_rows[(t + 1) % 2]
            # hop1 aggregation (this layer)
            scatter_hop(0, acc1, xsrc)
            # dense + epilogue per 128-node block
            for nb in range(NBN):
                b, half = nb // 2, nb % 2
                ps = densep.tile([P, P], F32, tag="psd", space="PSUM")
                nc.tensor.matmul(
                    out=ps[:],
                    lhsT=acc1[:, b, half * P:(half + 1) * P],
                    rhs=t_w1[:, t, :], start=True, stop=(t == 0))
                if t > 0:
                    nc.tensor.matmul(
                        out=ps[:],
                        lhsT=acc2[(t + 1) % 2][:, b, half * P:(half + 1) * P],
                        rhs=t_w2[:, t, :], start=False, stop=True)
                u = misc.tile([P, P], F32, tag="u")
                if has_bias:
                    nc.vector.tensor_tensor(
                        out=u[:], in0=ps[:], in1=t_bias[:, t, :],
                        op=mybir.AluOpType.add)
                    nc.vector.tensor_scalar_max(u[:], u[:], 0.0)
                else:
                    nc.vector.tensor_scalar_max(u[:], ps[:], 0.0)
                nc.vector.tensor_tensor(
                    out=xnew[:, nb, :], in0=u[:], in1=xcur[:, nb, :],
                    op=mybir.AluOpType.add)
                sq = misc.tile([P, P], F32, tag="sq")
                nc.scalar.activation(
                    out=sq[:], in_=xnew[:, nb, :],
                    func=mybir.ActivationFunctionType.Square,
                    accum_out=ssum[:, nb:nb + 1])
            nc.scalar.sqrt(sinv[:], ssum[:])
            nc.vector.tensor_scalar_max(sinv[:], sinv[:], 1e-12)
            nc.vector.reciprocal(sinv[:], sinv[:])
            for nb in range(NBN):
                nc.scalar.activation(
                    out=xnew[:, nb, :], in_=xnew[:, nb, :],
                    func=mybir.ActivationFunctionType.Copy,
                    scale=sinv[:, nb:nb + 1])
            if t < L - 1:
                nc.sync.dma_start(
                    ag_in[t][:].rearrange("(a p) f -> p a f", p=P), xnew[:])
                nc.gpsimd.collective_compute(
                    "AllGather", mybir.AluOpType.bypass,
                    ins=[ag_in[t][:]], outs=[ag_out[t][:]],
                    replica_groups=[list(range(NCORES))],
                )
                # hop2 aggregation for next layer; overlaps the AllGather
                scatter_hop(1, acc2[t % 2], xsrc)
            else:
                nc.sync.dma_start(
                    out_own[:].rearrange("(a p) f -> p a f", p=P), xnew[:])
    nc.compile()
    return nc


def _prepare(x, W1, b1, W2, b2, alpha, src1, dst1, src2, dst2):
    N, D = x.shape
    L = W1.shape[0]
    assert D == P
    nblk = -(-N // (NCORES * BLK))
    NP = nblk * BLK
    NPAD = NP * NCORES

    norm1 = _edge_norm(src1, dst1, N)
    norm2 = _edge_norm(src2, dst2, N)
    CA1, CB1, tabs1 = _prep_hop(src1, dst1, norm1, NP, nblk)
    CA2, CB2, tabs2 = _prep_hop(src2, dst2, norm2, NP, nblk)

    a = np.zeros((L, 2), np.float32)
    a[0] = [1.0, 0.0]
    for t in range(1, L):
        a[t] = _softmax(alpha[t].astype(np.float32))
    w1s = (W1 * a[:, 0, None, None]).astype(np.float32)
    w2s = (W2 * a[:, 1, None, None]).astype(np.float32)
    bias = (a[:, 0, None] * b1 + a[:, 1, None] * b2).astype(np.float32)
    bias_b = np.broadcast_to(bias[:, None, :], (L, P, P)).copy()

    xpad = np.zeros((NPAD, P), np.float32)
    xpad[:N] = x
    iota = np.tile(np.arange(BLK, dtype=np.float32), (P, 1)).copy()

    in_maps = []
    for k in range(NCORES):
        m = dict(
            x_full=xpad, x_own=xpad[k * NP:(k + 1) * NP],
            w1s=w1s, w2s=w2s, biasb=bias_b, iota=iota,
            dst1=tabs1[k][0], nrm1=tabs1[k][1], idxa1=tabs1[k][2],
            dst2=tabs2[k][0], nrm2=tabs2[k][1], idxa2=tabs2[k][2],
        )
        if tabs1[k][3] is not None:
            m["idxb1"] = tabs1[k][3]
        if tabs2[k][3] is not None:
            m["idxb2"] = tabs2[k][3]
        in_maps.append(m)
    has_bias = bool(np.any(bias))
    return nblk, (CA1, CB1, CA2, CB2), L, N, NP, has_bias, in_maps


_CACHE = {}


def run(x, W1, b1, W2, b2, alpha, src1, dst1, src2, dst2,
        msg_dt_name="float32r", trace=False):
    from concourse import bass_utils
    nblk, Cs, L, N, NP, has_bias, in_maps = _prepare(
        x, W1, b1, W2, b2, alpha, src1, dst1, src2, dst2)
    key = (nblk,) + Cs + (L, has_bias, msg_dt_name)
    if key not in _CACHE:
        _CACHE[key] = _build(nblk, *Cs, L, has_bias, msg_dt_name)
    nc = _CACHE[key]
    res = bass_utils.run_bass_kernel_spmd(
        nc, in_maps, core_ids=list(range(NCORES)), trace=trace)
    out = np.concatenate([res.results[k]["out_own"] for k in range(NCORES)],
                         axis=0)[:N]
    return out, res


def kernel(x, W1, b1, W2, b2, alpha, src1, dst1, src2, dst2):
    out, _ = run(np.asarray(x, np.float32), np.asarray(W1, np.float32),
                 np.asarray(b1, np.float32), np.asarray(W2, np.float32),
                 np.asarray(b2, np.float32), np.asarray(alpha, np.float32),
                 np.asarray(src1, np.int32), np.asarray(dst1, np.int32),
                 np.asarray(src2, np.int32), np.asarray(dst2, np.int32))
    return out



# revision 4
# speedup vs baseline: 1.0857x; 1.0857x over previous
"""DelayGNN stage kernel for 8 Trainium2 NeuronCores.

Strategy (graph/data parallel):
  - Nodes sharded across 8 cores (6400 padded nodes each); edge lists
    partitioned by destination core, sorted by (256-node destination block,
    table half, src), padded to uniform chunk counts so one SPMD program
    serves all cores.
  - bf16 message path: the node-feature gather table is bf16 (256B rows),
    scatter one-hots are built on DVE in bf16 (fast perf mode), and both
    scatter and dense matmuls run in bf16 with fp32 PSUM accumulation.
  - Gathers use int16 indices (table split in two halves for the int16
    range) and round-robin across 4 SWDGE queues so descriptor generation
    runs on all four Q7 core pairs concurrently (it is the kernel's
    critical resource). Trailing pad indices are -32768, which the Q7
    ucode trims (no descriptors, no wasted bandwidth).
  - Per layer: hop-1 scatter into per-block PSUM accumulators via one-hot
    matmuls, dense W matmuls, row-layout epilogue (relu + residual + L2
    normalize), then a bf16 AllGather of the new node features; the hop-2
    aggregation (only needed by the next layer) overlaps the AllGather.
"""
import os
import sys
import numpy as np

for _p in ("/opt/trn_rl_repo", "/root/.axon_site/_ro/trn_rl_repo"):
    if os.path.isdir(_p) and _p not in sys.path:
        sys.path.append(_p)

P = 128
BLK = 256
NCORES = 8
HALF = 32768      # int16 index ceiling per gather table half
MAXC = 8          # dma_gather descriptor-ring cap: <=1024 idxs per call
# SWDGE queues (Q7 core pairs) used for gathers
NQ = int(os.environ.get("KNQ", "4"))
# trailing pads; -32768 is trimmed by the Q7 ucode, 0 gathers row 0
PAD_IDX = int(os.environ.get("KPAD", "-32768"))


def _wrap_idx(flat):
    """[n] int -> dma_gather idx layout [128, n/16] (wrapped, replicated)."""
    n = len(flat)
    w = np.asarray(flat, np.int16).reshape(n // 16, 16).T  # [16, n/16]
    return np.ascontiguousarray(np.tile(w, (8, 1)))


def _prep_hop(src, dst, norm, n_per_core, nblk):
    """Partition edges by dst core, sort by (dst block, src half, src), pad
    each block to CA + CB chunks of 128. Returns (CA, CB, per-core tables)."""
    core = dst // n_per_core
    percore = []
    cntA = np.zeros(nblk, np.int64)
    cntB = np.zeros(nblk, np.int64)
    for k in range(NCORES):
        sel = core == k
        s, d, w = src[sel], dst[sel] - k * n_per_core, norm[sel]
        blk = d // BLK
        isB = (s >= HALF).astype(np.int64)
        order = np.lexsort((s, isB, blk))
        s, d, w, blk, isB = (a[order] for a in (s, d, w, blk, isB))
        grp = blk * 2 + isB
        cnt = np.bincount(grp, minlength=2 * nblk)
        starts = np.concatenate([[0], np.cumsum(cnt)[:-1]])
        rank = np.arange(len(s)) - starts[grp]
        percore.append((s, d, w, blk, isB, rank))
        cntA = np.maximum(cntA, cnt[0::2])
        cntB = np.maximum(cntB, cnt[1::2])
    # per-block chunk counts (max over cores -> SPMD-uniform program)
    CAb = np.maximum(1, -(-cntA // P)).astype(np.int64)
    CBb = (-(-cntB // P)).astype(np.int64)
    Cgb = CAb + CBb
    choff = np.concatenate([[0], np.cumsum(Cgb)])       # chunk offsets
    aoff = np.concatenate([[0], np.cumsum(CAb)])        # A-chunk offsets
    boff = np.concatenate([[0], np.cumsum(CBb)])        # B-chunk offsets
    J = int(Cgb.sum())
    out = []
    for k in range(NCORES):
        s, d, w, blk, isB, rank = percore[k]
        gix = np.full(J * P, PAD_IDX, np.int64)
        dp = np.zeros(J * P, np.float32)
        wp = np.zeros(J * P, np.float32)
        pos = choff[blk] * P + isB * (CAb[blk] * P) + rank
        gix[pos] = np.where(isB == 1, s - HALF, s)
        dp[pos] = (d % BLK).astype(np.float32)
        wp[pos] = w
        idxA = np.concatenate(
            [_wrap_idx(gix[choff[b] * P:(choff[b] + CAb[b]) * P])
             for b in range(nblk)], axis=1)
        idxB = (np.concatenate(
            [_wrap_idx(gix[(choff[b] + CAb[b]) * P:choff[b + 1] * P])
             for b in range(nblk) if CBb[b]], axis=1)
            if CBb.sum() else None)
        out.append((
            np.ascontiguousarray(dp.reshape(-1, P).T),
            np.ascontiguousarray(wp.reshape(-1, P).T),
            idxA, idxB,
        ))
    return tuple(CAb), tuple(CBb), out


def _edge_norm(src, dst, n):
    ones = np.ones(len(src), np.float32)
    deg_out = np.bincount(src, weights=ones, minlength=n).astype(np.float32)
    deg_in = np.bincount(dst, weights=ones, minlength=n).astype(np.float32)
    inv_out = np.where(deg_out > 0,
                       (1.0 / np.sqrt(np.maximum(deg_out, 1.0))), 0.0)
    inv_in = np.where(deg_in > 0,
                      (1.0 / np.sqrt(np.maximum(deg_in, 1.0))), 0.0)
    return (inv_out[src] * inv_in[dst]).astype(np.float32)


def _softmax(v):
    e = np.exp(v - v.max())
    return (e / e.sum()).astype(np.float32)


def _build(nblk, CA1, CB1, CA2, CB2, L, has_bias):
    """Build the SPMD Bass program. nblk 256-dst blocks per core."""
    import concourse.bass as bass
    import concourse.tile as tile
    from concourse import bacc, mybir
    from concourse.library_config import mlp
    from contextlib import ExitStack

    F32 = mybir.dt.float32
    BF16 = mybir.dt.bfloat16
    I16 = mybir.dt.int16
    NP = nblk * BLK            # nodes per core
    NPAD = NP * NCORES
    HB = min(HALF, NPAD)       # rows in table half A
    NBN = NP // P              # 128-node blocks per core
    CAs, CBs = (np.asarray(CA1), np.asarray(CA2)), (np.asarray(CB1),
                                                     np.asarray(CB2))
    choffs = [np.concatenate([[0], np.cumsum(CAs[h] + CBs[h])])
              for h in range(2)]
    aoffs = [np.concatenate([[0], np.cumsum(CAs[h])]) for h in range(2)]
    boffs = [np.concatenate([[0], np.cumsum(CBs[h])]) for h in range(2)]
    Js = (int(choffs[0][-1]), int(choffs[1][-1]))

    nc = bacc.Bacc("TRN2", target_bir_lowering=False, debug=False,
                   num_devices=NCORES, num_swdge_queues=NQ)

    x_bf = nc.dram_tensor("x_bf", [NPAD, P], BF16, kind="ExternalInput")
    x_own = nc.dram_tensor("x_own", [NP, P], F32, kind="ExternalInput")
    w1d = nc.dram_tensor("w1s", [L, P, P], BF16, kind="ExternalInput")
    w2d = nc.dram_tensor("w2s", [L, P, P], BF16, kind="ExternalInput")
    iotad = nc.dram_tensor("iota", [P, BLK], BF16, kind="ExternalInput")
    dstd = [nc.dram_tensor(f"dst{h}", [P, J], F32, kind="ExternalInput")
            for h, J in ((1, Js[0]), (2, Js[1]))]
    nrmd = [nc.dram_tensor(f"nrm{h}", [P, J], F32, kind="ExternalInput")
            for h, J in ((1, Js[0]), (2, Js[1]))]
    idxad = [nc.dram_tensor(f"idxa{h+1}", [P, int(aoffs[h][-1]) * 8], I16,
                            kind="ExternalInput") for h in range(2)]
    idxbd = [nc.dram_tensor(f"idxb{h+1}", [P, int(boffs[h][-1]) * 8], I16,
                            kind="ExternalInput") if CBs[h].sum() else None
             for h in range(2)]
    if has_bias:
        biasd = nc.dram_tensor("biasb", [L, P, P], F32, kind="ExternalInput")
    out_own = nc.dram_tensor("out_own", [NP, P], F32, kind="ExternalOutput")

    ag_in = [nc.dram_tensor(f"ag_in{t}", [NP, P], BF16, kind="Internal")
             for t in range(L - 1)]
    ag_out = [nc.dram_tensor(f"ag_out{t}", [NPAD, P], BF16, kind="Internal",
                             addr_space="Shared")
              for t in range(L - 1)]

    with tile.TileContext(nc) as tc, ExitStack() as ctx:
        sb = ctx.enter_context(tc.tile_pool(name="sb", bufs=1))
        gq = [ctx.enter_context(tc.tile_pool(name=f"g{q}", bufs=3))
              for q in range(NQ)]
        ohpool = ctx.enter_context(tc.tile_pool(name="oh", bufs=4))
        accp = ctx.enter_context(
            tc.tile_pool(name="accp", bufs=2, space="PSUM"))
        densep = ctx.enter_context(
            tc.tile_pool(name="densep", bufs=2, space="PSUM"))
        misc = ctx.enter_context(tc.tile_pool(name="misc", bufs=2))

        # --- persistent tiles ---
        t_dst = [sb.tile([P, Js[0]], F32, tag="dst1", name="tdst1"),
                 sb.tile([P, Js[1]], F32, tag="dst2", name="tdst2")]
        t_nrm = [sb.tile([P, Js[0]], F32, tag="nrm1", name="tnrm1"),
                 sb.tile([P, Js[1]], F32, tag="nrm2", name="tnrm2")]
        t_ixa = [sb.tile([P, int(aoffs[h][-1]) * 8], I16, tag=f"ixa{h}",
                         name=f"ixa{h}") for h in range(2)]
        t_ixb = [sb.tile([P, int(boffs[h][-1]) * 8], I16, tag=f"ixb{h}",
                         name=f"ixb{h}") if CBs[h].sum() else None
                 for h in range(2)]
        t_iota = sb.tile([P, BLK], BF16, tag="iota")
        t_w1 = sb.tile([P, L, P], BF16, tag="w1")
        t_w2 = sb.tile([P, L, P], BF16, tag="w2")
        if has_bias:
            t_bias = sb.tile([P, L, P], F32, tag="bias")
        x_rows = [sb.tile([P, NBN, P], F32, tag=f"xr{i}", name=f"xr{i}")
                  for i in range(2)]
        xbf = sb.tile([P, NBN, P], BF16, tag="xbf")
        acc1 = sb.tile([P, nblk, BLK], BF16, tag="acc1")
        acc2 = [sb.tile([P, nblk, BLK], BF16, tag=f"acc2_{i}",
                        name=f"acc2_{i}") for i in range(2)]
        ssum = sb.tile([P, NBN], F32, tag="ssum")
        sinv = sb.tile([P, NBN], F32, tag="sinv")

        nc.gpsimd.load_library(mlp)
        for h in range(2):
            nc.sync.dma_start(t_dst[h][:], dstd[h][:])
            nc.sync.dma_start(t_nrm[h][:], nrmd[h][:])
            nc.sync.dma_start(t_ixa[h][:], idxad[h][:])
            if CBs[h].sum():
                nc.sync.dma_start(t_ixb[h][:], idxbd[h][:])
        nc.sync.dma_start(t_iota[:], iotad[:])
        nc.sync.dma_start(t_w1[:], w1d[:].rearrange("t i o -> i t o"))
        nc.sync.dma_start(t_w2[:], w2d[:].rearrange("t i o -> i t o"))
        if has_bias:
            nc.sync.dma_start(t_bias[:], biasd[:].rearrange("t i o -> i t o"))
        nc.sync.dma_start(x_rows[0][:],
                          x_own[:].rearrange("(a p) f -> p a f", p=P))

        qrr = [0]  # gather queue round-robin state

        def gather_pieces(tab_ap, idx_tile, ch0, C):
            """Gather C chunks (idx-table chunk offset ch0) in pieces of
            <=MAXC chunks. Returns [(first_chunk, piece_tile)]."""
            pieces = []
            for p0 in range(0, C, MAXC):
                pc = min(MAXC, C - p0)
                q = qrr[0] % NQ
                qrr[0] += 1
                ni = pc * P
                xg = gq[q].tile([P, MAXC, P], BF16, tag=f"xg{q}",
                                name=f"xg{q}")
                col0 = (ch0 + p0) * 8
                nc.gpsimd.dma_gather(
                    out_ap=xg[:, 0:pc, :], in_ap=tab_ap,
                    idxs_ap=idx_tile[:, col0:col0 + pc * 8],
                    num_idxs=ni, num_idxs_reg=ni, elem_size=P,
                    queue_num=q)
                pieces.append((p0, xg))
            return pieces

        def scatter_hop(h, acc_sb, tab):
            for b in range(nblk):
                CA, CB = int(CAs[h][b]), int(CBs[h][b])
                Cg = CA + CB
                pa = gather_pieces(tab[0:HB, :], t_ixa[h],
                                   int(aoffs[h][b]), CA)
                pb = (gather_pieces(tab[HB:NPAD, :], t_ixb[h],
                                    int(boffs[h][b]), CB)
                      if CB else [])

                def chunk_ap(c):
                    pieces, cc = (pa, c) if c < CA else (pb, c - CA)
                    for p0, xg in reversed(pieces):
                        if cc >= p0:
                            return xg[:, cc - p0, :]
                    raise AssertionError

                ps = accp.tile([P, BLK], F32, tag="psacc", space="PSUM")
                for c in range(Cg):
                    j = int(choffs[h][b]) + c
                    xsl = chunk_ap(c)
                    oh = ohpool.tile([P, BLK], BF16, tag="oh")
                    nc.vector.tensor_scalar(
                        out=oh[:], in0=t_iota[:],
                        scalar1=t_dst[h][:, j:j + 1],
                        scalar2=t_nrm[h][:, j:j + 1],
                        op0=mybir.AluOpType.is_equal,
                        op1=mybir.AluOpType.mult,
                    )
                    nc.tensor.matmul(out=ps[:], lhsT=xsl, rhs=oh[:],
                                     start=(c == 0), stop=(c == Cg - 1))
                nc.scalar.copy(acc_sb[:, b, :], ps[:])

        for t in range(L):
            tab = x_bf[:] if t == 0 else ag_out[t - 1][:]
            xcur = x_rows[t % 2]
            xnew = x_rows[(t + 1) % 2]
            # hop1 aggregation (this layer)
            scatter_hop(0, acc1, tab)
            # dense + epilogue per 128-node block
            for nb in range(NBN):
                b, hf = nb // 2, nb % 2
                ps = densep.tile([P, P], F32, tag="psd", space="PSUM")
                nc.tensor.matmul(
                    out=ps[:],
                    lhsT=acc1[:, b, hf * P:(hf + 1) * P],
                    rhs=t_w1[:, t, :], start=True, stop=(t == 0))
                if t > 0:
                    nc.tensor.matmul(
                        out=ps[:],
                        lhsT=acc2[(t + 1) % 2][:, b, hf * P:(hf + 1) * P],
                        rhs=t_w2[:, t, :], start=False, stop=True)
                u = misc.tile([P, P], F32, tag="u")
                if has_bias:
                    nc.vector.tensor_tensor(
                        out=u[:], in0=ps[:], in1=t_bias[:, t, :],
                        op=mybir.AluOpType.add)
                    nc.vector.tensor_scalar_max(u[:], u[:], 0.0)
                else:
                    nc.scalar.activation(
                        out=u[:], in_=ps[:],
                        func=mybir.ActivationFunctionType.Relu)
                nc.vector.tensor_tensor(
                    out=xnew[:, nb, :], in0=u[:], in1=xcur[:, nb, :],
                    op=mybir.AluOpType.add)
                sq = misc.tile([P, P], F32, tag="sq")
                nc.scalar.activation(
                    out=sq[:], in_=xnew[:, nb, :],
                    func=mybir.ActivationFunctionType.Square,
                    accum_out=ssum[:, nb:nb + 1])
            nc.scalar.sqrt(sinv[:], ssum[:])
            nc.vector.tensor_scalar_max(sinv[:], sinv[:], 1e-12)
            nc.vector.reciprocal(sinv[:], sinv[:])
            for nb in range(NBN):
                if t < L - 1:
                    nc.vector.tensor_scalar_mul(
                        xbf[:, nb, :], xnew[:, nb, :], sinv[:, nb:nb + 1])
                nc.scalar.activation(
                    out=xnew[:, nb, :], in_=xnew[:, nb, :],
                    func=mybir.ActivationFunctionType.Copy,
                    scale=sinv[:, nb:nb + 1])
            if t < L - 1:
                nc.sync.dma_start(
                    ag_in[t][:].rearrange("(a p) f -> p a f", p=P), xbf[:])
                nc.gpsimd.collective_compute(
                    "AllGather", mybir.AluOpType.bypass,
                    ins=[ag_in[t][:]], outs=[ag_out[t][:]],
                    replica_groups=[list(range(NCORES))],
                )
                # hop2 aggregation for next layer; overlaps the AllGather
                scatter_hop(1, acc2[t % 2], tab)
            else:
                nc.sync.dma_start(
                    out_own[:].rearrange("(a p) f -> p a f", p=P), xnew[:])
    nc.compile()
    return nc


def _prepare(x, W1, b1, W2, b2, alpha, src1, dst1, src2, dst2):
    import ml_dtypes
    BF = ml_dtypes.bfloat16
    N, D = x.shape
    L = W1.shape[0]
    assert D == P
    nblk = -(-N // (NCORES * BLK))
    NP = nblk * BLK
    NPAD = NP * NCORES

    norm1 = _edge_norm(src1, dst1, N)
    norm2 = _edge_norm(src2, dst2, N)
    CA1, CB1, tabs1 = _prep_hop(src1, dst1, norm1, NP, nblk)
    CA2, CB2, tabs2 = _prep_hop(src2, dst2, norm2, NP, nblk)

    a = np.zeros((L, 2), np.float32)
    a[0] = [1.0, 0.0]
    for t in range(1, L):
        a[t] = _softmax(alpha[t].astype(np.float32))
    w1s = (W1 * a[:, 0, None, None]).astype(BF)
    w2s = (W2 * a[:, 1, None, None]).astype(BF)
    bias = (a[:, 0, None] * b1 + a[:, 1, None] * b2).astype(np.float32)
    bias_b = np.broadcast_to(bias[:, None, :], (L, P, P)).copy()

    xpad = np.zeros((NPAD, P), np.float32)
    xpad[:N] = x
    xpad_bf = xpad.astype(BF)
    iota = np.tile(np.arange(BLK, dtype=np.float32), (P, 1)).astype(BF)

    in_maps = []
    for k in range(NCORES):
        m = dict(
            x_bf=xpad_bf, x_own=xpad[k * NP:(k + 1) * NP],
            w1s=w1s, w2s=w2s, iota=iota,
            dst1=tabs1[k][0], nrm1=tabs1[k][1], idxa1=tabs1[k][2],
            dst2=tabs2[k][0], nrm2=tabs2[k][1], idxa2=tabs2[k][2],
        )
        if tabs1[k][3] is not None:
            m["idxb1"] = tabs1[k][3]
        if tabs2[k][3] is not None:
            m["idxb2"] = tabs2[k][3]
        if np.any(bias):
            m["biasb"] = bias_b
        in_maps.append(m)
    has_bias = bool(np.any(bias))
    return nblk, (CA1, CB1, CA2, CB2), L, N, NP, has_bias, in_maps


_CACHE = {}


def run(x, W1, b1, W2, b2, alpha, src1, dst1, src2, dst2,
        msg_dt_name="bfloat16", trace=False):
    from concourse import bass_utils
    nblk, Cs, L, N, NP, has_bias, in_maps = _prepare(
        x, W1, b1, W2, b2, alpha, src1, dst1, src2, dst2)
    key = (nblk,) + Cs + (L, has_bias)
    if key not in _CACHE:
        _CACHE[key] = _build(nblk, *Cs, L, has_bias)
    nc = _CACHE[key]
    res = bass_utils.run_bass_kernel_spmd(
        nc, in_maps, core_ids=list(range(NCORES)), trace=trace)
    out = np.concatenate([res.results[k]["out_own"] for k in range(NCORES)],
                         axis=0)[:N]
    return out, res


def kernel(x, W1, b1, W2, b2, alpha, src1, dst1, src2, dst2):
    out, _ = run(np.asarray(x, np.float32), np.asarray(W1, np.float32),
                 np.asarray(b1, np.float32), np.asarray(W2, np.float32),
                 np.asarray(b2, np.float32), np.asarray(alpha, np.float32),
                 np.asarray(src1, np.int32), np.asarray(dst1, np.int32),
                 np.asarray(src2, np.int32), np.asarray(dst2, np.int32))
    return out


# revision 8
# speedup vs baseline: 1.1999x; 1.1052x over previous
"""DelayGNN stage kernel for 8 Trainium2 NeuronCores.

Strategy (graph/data parallel):
  - Nodes sharded across 8 cores (6400 padded nodes each); edge lists
    partitioned by destination core, sorted by (256-node destination block,
    table half, src), padded to uniform chunk counts so one SPMD program
    serves all cores.
  - bf16 message path: the node-feature gather table is bf16 (256B rows),
    scatter one-hots are built on DVE in bf16 (fast perf mode), and both
    scatter and dense matmuls run in bf16 with fp32 PSUM accumulation.
  - Gathers use int16 indices (table split in two halves for the int16
    range) and round-robin across 4 SWDGE queues so descriptor generation
    runs on all four Q7 core pairs concurrently (it is the kernel's
    critical resource). Trailing pad indices are -32768, which the Q7
    ucode trims (no descriptors, no wasted bandwidth).
  - Per layer: hop-1 scatter into per-block PSUM accumulators via one-hot
    matmuls, dense W matmuls, row-layout epilogue (relu + residual + L2
    normalize), then a bf16 AllGather of the new node features; the hop-2
    aggregation (only needed by the next layer) overlaps the AllGather.
"""
import os
import sys
import numpy as np

for _p in ("/opt/trn_rl_repo", "/root/.axon_site/_ro/trn_rl_repo"):
    if os.path.isdir(_p) and _p not in sys.path:
        sys.path.append(_p)

P = 128
BLK = 256
NCORES = 8
HALF = 32768      # int16 index ceiling per gather table half
MAXC = 8          # dma_gather descriptor-ring cap: <=1024 idxs per call
# SWDGE queues (Q7 core pairs) used for gathers
NQ = int(os.environ.get("KNQ", "4"))
# pad idx 0 gathers row 0 (harmless; its one-hot weight is 0). Calls span
# block boundaries, so pads are interior and must be valid indices.
PAD_IDX = int(os.environ.get("KPAD", "0"))
GBUFS = int(os.environ.get("KGBUFS", "6"))


def _wrap_idx(flat):
    """[n] int -> dma_gather idx layout [128, n/16] (wrapped, replicated)."""
    n = len(flat)
    w = np.asarray(flat, np.int16).reshape(n // 16, 16).T  # [16, n/16]
    return np.ascontiguousarray(np.tile(w, (8, 1)))


def _prep_hop(src, dst, norm, n_per_core, nblk):
    """Partition edges by dst core, sort by (dst block, src half, src), pad
    each block to CA + CB chunks of 128. Returns (CA, CB, per-core tables)."""
    core = dst // n_per_core
    percore = []
    cntA = np.zeros(nblk, np.int64)
    cntB = np.zeros(nblk, np.int64)
    for k in range(NCORES):
        sel = core == k
        s, d, w = src[sel], dst[sel] - k * n_per_core, norm[sel]
        blk = d // BLK
        isB = (s >= HALF).astype(np.int64)
        order = np.lexsort((s, isB, blk))
        s, d, w, blk, isB = (a[order] for a in (s, d, w, blk, isB))
        grp = blk * 2 + isB
        cnt = np.bincount(grp, minlength=2 * nblk)
        starts = np.concatenate([[0], np.cumsum(cnt)[:-1]])
        rank = np.arange(len(s)) - starts[grp]
        percore.append((s, d, w, blk, isB, rank))
        cntA = np.maximum(cntA, cnt[0::2])
        cntB = np.maximum(cntB, cnt[1::2])
    # per-block chunk counts (max over cores -> SPMD-uniform program)
    CAb = np.maximum(1, -(-cntA // P)).astype(np.int64)
    CBb = (-(-cntB // P)).astype(np.int64)
    Cgb = CAb + CBb
    choff = np.concatenate([[0], np.cumsum(Cgb)])       # chunk offsets
    aoff = np.concatenate([[0], np.cumsum(CAb)])        # A-chunk offsets
    boff = np.concatenate([[0], np.cumsum(CBb)])        # B-chunk offsets
    J = int(Cgb.sum())
    out = []
    for k in range(NCORES):
        s, d, w, blk, isB, rank = percore[k]
        gix = np.full(J * P, PAD_IDX, np.int64)
        dp = np.zeros(J * P, np.float32)
        wp = np.zeros(J * P, np.float32)
        pos = choff[blk] * P + isB * (CAb[blk] * P) + rank
        gix[pos] = np.where(isB == 1, s - HALF, s)
        dp[pos] = (d % BLK).astype(np.float32)
        wp[pos] = w
        idxA = np.concatenate(
            [_wrap_idx(gix[choff[b] * P:(choff[b] + CAb[b]) * P])
             for b in range(nblk)], axis=1)
        idxB = (np.concatenate(
            [_wrap_idx(gix[(choff[b] + CAb[b]) * P:choff[b + 1] * P])
             for b in range(nblk) if CBb[b]], axis=1)
            if CBb.sum() else None)
        out.append((
            np.ascontiguousarray(dp.reshape(-1, P).T),
            np.ascontiguousarray(wp.reshape(-1, P).T),
            idxA, idxB,
        ))
    return tuple(CAb), tuple(CBb), out


def _edge_norm(src, dst, n):
    ones = np.ones(len(src), np.float32)
    deg_out = np.bincount(src, weights=ones, minlength=n).astype(np.float32)
    deg_in = np.bincount(dst, weights=ones, minlength=n).astype(np.float32)
    inv_out = np.where(deg_out > 0,
                       (1.0 / np.sqrt(np.maximum(deg_out, 1.0))), 0.0)
    inv_in = np.where(deg_in > 0,
                      (1.0 / np.sqrt(np.maximum(deg_in, 1.0))), 0.0)
    return (inv_out[src] * inv_in[dst]).astype(np.float32)


def _softmax(v):
    e = np.exp(v - v.max())
    return (e / e.sum()).astype(np.float32)


def _build(nblk, CA1, CB1, CA2, CB2, L, has_bias):
    """Build the SPMD Bass program. nblk 256-dst blocks per core."""
    import concourse.bass as bass
    import concourse.tile as tile
    from concourse import bacc, mybir
    from concourse.library_config import mlp
    from contextlib import ExitStack

    F32 = mybir.dt.float32
    BF16 = mybir.dt.bfloat16
    I16 = mybir.dt.int16
    NP = nblk * BLK            # nodes per core
    NPAD = NP * NCORES
    HB = min(HALF, NPAD)       # rows in table half A
    NBN = NP // P              # 128-node blocks per core
    CAs, CBs = (np.asarray(CA1), np.asarray(CA2)), (np.asarray(CB1),
                                                     np.asarray(CB2))
    choffs = [np.concatenate([[0], np.cumsum(CAs[h] + CBs[h])])
              for h in range(2)]
    aoffs = [np.concatenate([[0], np.cumsum(CAs[h])]) for h in range(2)]
    boffs = [np.concatenate([[0], np.cumsum(CBs[h])]) for h in range(2)]
    Js = (int(choffs[0][-1]), int(choffs[1][-1]))

    nc = bacc.Bacc("TRN2", target_bir_lowering=False, debug=False,
                   num_devices=NCORES, num_swdge_queues=NQ)

    x_bf = nc.dram_tensor("x_bf", [NPAD, P], BF16, kind="ExternalInput")
    x_own = nc.dram_tensor("x_own", [NP, P], F32, kind="ExternalInput")
    w1d = nc.dram_tensor("w1s", [L, P, P], BF16, kind="ExternalInput")
    w2d = nc.dram_tensor("w2s", [L, P, P], BF16, kind="ExternalInput")
    iotad = nc.dram_tensor("iota", [P, BLK], BF16, kind="ExternalInput")
    dstd = [nc.dram_tensor(f"dst{h}", [P, J], F32, kind="ExternalInput")
            for h, J in ((1, Js[0]), (2, Js[1]))]
    nrmd = [nc.dram_tensor(f"nrm{h}", [P, J], F32, kind="ExternalInput")
            for h, J in ((1, Js[0]), (2, Js[1]))]
    idxad = [nc.dram_tensor(f"idxa{h+1}", [P, int(aoffs[h][-1]) * 8], I16,
                            kind="ExternalInput") for h in range(2)]
    idxbd = [nc.dram_tensor(f"idxb{h+1}", [P, int(boffs[h][-1]) * 8], I16,
                            kind="ExternalInput") if CBs[h].sum() else None
             for h in range(2)]
    if has_bias:
        biasd = nc.dram_tensor("biasb", [L, P, P], F32, kind="ExternalInput")
    out_own = nc.dram_tensor("out_own", [NP, P], F32, kind="ExternalOutput")

    ag_in = [nc.dram_tensor(f"ag_in{t}", [NP, P], BF16, kind="Internal")
             for t in range(L - 1)]
    ag_out = [nc.dram_tensor(f"ag_out{t}", [NPAD, P], BF16, kind="Internal",
                             addr_space="Shared")
              for t in range(L - 1)]

    with tile.TileContext(nc) as tc, ExitStack() as ctx:
        sb = ctx.enter_context(tc.tile_pool(name="sb", bufs=1))
        gq = [ctx.enter_context(tc.tile_pool(name=f"g{q}", bufs=GBUFS))
              for q in range(NQ)]
        ohpool = ctx.enter_context(tc.tile_pool(name="oh", bufs=4))
        accp = ctx.enter_context(
            tc.tile_pool(name="accp", bufs=2, space="PSUM"))
        densep = ctx.enter_context(
            tc.tile_pool(name="densep", bufs=2, space="PSUM"))
        misc = ctx.enter_context(tc.tile_pool(name="misc", bufs=2))

        # --- persistent tiles ---
        t_dst = [sb.tile([P, Js[0]], F32, tag="dst1", name="tdst1"),
                 sb.tile([P, Js[1]], F32, tag="dst2", name="tdst2")]
        t_nrm = [sb.tile([P, Js[0]], F32, tag="nrm1", name="tnrm1"),
                 sb.tile([P, Js[1]], F32, tag="nrm2", name="tnrm2")]
        t_ixa = [sb.tile([P, int(aoffs[h][-1]) * 8], I16, tag=f"ixa{h}",
                         name=f"ixa{h}") for h in range(2)]
        t_ixb = [sb.tile([P, int(boffs[h][-1]) * 8], I16, tag=f"ixb{h}",
                         name=f"ixb{h}") if CBs[h].sum() else None
                 for h in range(2)]
        t_iota = sb.tile([P, BLK], BF16, tag="iota")
        t_w1 = sb.tile([P, L, P], BF16, tag="w1")
        t_w2 = sb.tile([P, L, P], BF16, tag="w2")
        if has_bias:
            t_bias = sb.tile([P, L, P], F32, tag="bias")
        x_rows = [sb.tile([P, NBN, P], F32, tag=f"xr{i}", name=f"xr{i}")
                  for i in range(2)]
        xbf = sb.tile([P, NBN, P], BF16, tag="xbf")
        acc1 = sb.tile([P, nblk, BLK], BF16, tag="acc1")
        acc2 = [sb.tile([P, nblk, BLK], BF16, tag=f"acc2_{i}",
                        name=f"acc2_{i}") for i in range(2)]
        ssum = sb.tile([P, NBN], F32, tag="ssum")
        sinv = sb.tile([P, NBN], F32, tag="sinv")

        nc.gpsimd.load_library(mlp)
        for h in range(2):
            nc.sync.dma_start(t_dst[h][:], dstd[h][:])
            nc.sync.dma_start(t_nrm[h][:], nrmd[h][:])
            nc.sync.dma_start(t_ixa[h][:], idxad[h][:])
            if CBs[h].sum():
                nc.sync.dma_start(t_ixb[h][:], idxbd[h][:])
        nc.sync.dma_start(t_iota[:], iotad[:])
        nc.sync.dma_start(t_w1[:], w1d[:].rearrange("t i o -> i t o"))
        nc.sync.dma_start(t_w2[:], w2d[:].rearrange("t i o -> i t o"))
        if has_bias:
            nc.sync.dma_start(t_bias[:], biasd[:].rearrange("t i o -> i t o"))
        nc.sync.dma_start(x_rows[0][:],
                          x_own[:].rearrange("(a p) f -> p a f", p=P))

        qrr = [0]  # gather queue round-robin state

        def gather_pieces(tab_ap, idx_tile, ch0, C):
            """Gather C chunks (idx-table chunk offset ch0) in pieces of
            <=MAXC chunks. Returns [(first_chunk, piece_tile)]."""
            pieces = []
            for p0 in range(0, C, MAXC):
                pc = min(MAXC, C - p0)
                q = qrr[0] % NQ
                qrr[0] += 1
                ni = pc * P
                xg = gq[q].tile([P, MAXC, P], BF16, tag=f"xg{q}",
                                name=f"xg{q}")
                col0 = (ch0 + p0) * 8
                nc.gpsimd.dma_gather(
                    out_ap=xg[:, 0:pc, :], in_ap=tab_ap,
                    idxs_ap=idx_tile[:, col0:col0 + pc * 8],
                    num_idxs=ni, num_idxs_reg=ni, elem_size=P,
                    queue_num=q)
                pieces.append((p0, xg))
            return pieces

        def scatter_hop(h, acc_sb, tab):
            for b in range(nblk):
                CA, CB = int(CAs[h][b]), int(CBs[h][b])
                Cg = CA + CB
                pa = gather_pieces(tab[0:HB, :], t_ixa[h],
                                   int(aoffs[h][b]), CA)
                pb = (gather_pieces(tab[HB:NPAD, :], t_ixb[h],
                                    int(boffs[h][b]), CB)
                      if CB else [])

                def chunk_ap(c):
                    pieces, cc = (pa, c) if c < CA else (pb, c - CA)
                    for p0, xg in reversed(pieces):
                        if cc >= p0:
                            return xg[:, cc - p0, :]
                    raise AssertionError

                ps = accp.tile([P, BLK], F32, tag="psacc", space="PSUM")
                for c in range(Cg):
                    j = int(choffs[h][b]) + c
                    xsl = chunk_ap(c)
                    oh = ohpool.tile([P, BLK], BF16, tag="oh")
                    nc.vector.tensor_scalar(
                        out=oh[:], in0=t_iota[:],
                        scalar1=t_dst[h][:, j:j + 1],
                        scalar2=t_nrm[h][:, j:j + 1],
                        op0=mybir.AluOpType.is_equal,
                        op1=mybir.AluOpType.mult,
                    )
                    nc.tensor.matmul(out=ps[:], lhsT=xsl, rhs=oh[:],
                                     start=(c == 0), stop=(c == Cg - 1))
                nc.scalar.copy(acc_sb[:, b, :], ps[:])

        for t in range(L):
            tab = x_bf[:] if t == 0 else ag_out[t - 1][:]
            xcur = x_rows[t % 2]
            xnew = x_rows[(t + 1) % 2]
            # hop1 aggregation (this layer)
            scatter_hop(0, acc1, tab)
            # dense + epilogue per 128-node block
            for nb in range(NBN):
                b, hf = nb // 2, nb % 2
                ps = densep.tile([P, P], F32, tag="psd", space="PSUM")
                nc.tensor.matmul(
                    out=ps[:],
                    lhsT=acc1[:, b, hf * P:(hf + 1) * P],
                    rhs=t_w1[:, t, :], start=True, stop=(t == 0))
                if t > 0:
                    nc.tensor.matmul(
                        out=ps[:],
                        lhsT=acc2[(t + 1) % 2][:, b, hf * P:(hf + 1) * P],
                        rhs=t_w2[:, t, :], start=False, stop=True)
                u = misc.tile([P, P], F32, tag="u")
                if has_bias:
                    nc.vector.tensor_tensor(
                        out=u[:], in0=ps[:], in1=t_bias[:, t, :],
                        op=mybir.AluOpType.add)
                    nc.vector.tensor_scalar_max(u[:], u[:], 0.0)
                else:
                    nc.scalar.activation(
                        out=u[:], in_=ps[:],
                        func=mybir.ActivationFunctionType.Relu)
                nc.vector.tensor_tensor(
                    out=xnew[:, nb, :], in0=u[:], in1=xcur[:, nb, :],
                    op=mybir.AluOpType.add)
                sq = misc.tile([P, P], F32, tag="sq")
                nc.scalar.activation(
                    out=sq[:], in_=xnew[:, nb, :],
                    func=mybir.ActivationFunctionType.Square,
                    accum_out=ssum[:, nb:nb + 1])
            nc.scalar.sqrt(sinv[:], ssum[:])
            nc.vector.tensor_scalar_max(sinv[:], sinv[:], 1e-12)
            nc.vector.reciprocal(sinv[:], sinv[:])
            for nb in range(NBN):
                if t < L - 1:
                    nc.vector.tensor_scalar_mul(
                        xbf[:, nb, :], xnew[:, nb, :], sinv[:, nb:nb + 1])
                nc.scalar.activation(
                    out=xnew[:, nb, :], in_=xnew[:, nb, :],
                    func=mybir.ActivationFunctionType.Copy,
                    scale=sinv[:, nb:nb + 1])
            if t < L - 1:
                nc.sync.dma_start(
                    ag_in[t][:].rearrange("(a p) f -> p a f", p=P), xbf[:])
                nc.gpsimd.collective_compute(
                    "AllGather", mybir.AluOpType.bypass,
                    ins=[ag_in[t][:]], outs=[ag_out[t][:]],
                    replica_groups=[list(range(NCORES))],
                )
                # hop2 aggregation for next layer; overlaps the AllGather
                scatter_hop(1, acc2[t % 2], tab)
            else:
                nc.sync.dma_start(
                    out_own[:].rearrange("(a p) f -> p a f", p=P), xnew[:])
    nc.compile()
    return nc


def _prepare(x, W1, b1, W2, b2, alpha, src1, dst1, src2, dst2):
    import ml_dtypes
    BF = ml_dtypes.bfloat16
    N, D = x.shape
    L = W1.shape[0]
    assert D == P
    nblk = -(-N // (NCORES * BLK))
    NP = nblk * BLK
    NPAD = NP * NCORES

    norm1 = _edge_norm(src1, dst1, N)
    norm2 = _edge_norm(src2, dst2, N)
    CA1, CB1, tabs1 = _prep_hop(src1, dst1, norm1, NP, nblk)
    CA2, CB2, tabs2 = _prep_hop(src2, dst2, norm2, NP, nblk)

    a = np.zeros((L, 2), np.float32)
    a[0] = [1.0, 0.0]
    for t in range(1, L):
        a[t] = _softmax(alpha[t].astype(np.float32))
    w1s = (W1 * a[:, 0, None, None]).astype(BF)
    w2s = (W2 * a[:, 1, None, None]).astype(BF)
    bias = (a[:, 0, None] * b1 + a[:, 1, None] * b2).astype(np.float32)
    bias_b = np.broadcast_to(bias[:, None, :], (L, P, P)).copy()

    xpad = np.zeros((NPAD, P), np.float32)
    xpad[:N] = x
    xpad_bf = xpad.astype(BF)
    iota = np.tile(np.arange(BLK, dtype=np.float32), (P, 1)).astype(BF)

    in_maps = []
    for k in range(NCORES):
        m = dict(
            x_bf=xpad_bf, x_own=xpad[k * NP:(k + 1) * NP],
            w1s=w1s, w2s=w2s, iota=iota,
            dst1=tabs1[k][0], nrm1=tabs1[k][1], idxa1=tabs1[k][2],
            dst2=tabs2[k][0], nrm2=tabs2[k][1], idxa2=tabs2[k][2],
        )
        if tabs1[k][3] is not None:
            m["idxb1"] = tabs1[k][3]
        if tabs2[k][3] is not None:
            m["idxb2"] = tabs2[k][3]
        if np.any(bias):
            m["biasb"] = bias_b
        in_maps.append(m)
    has_bias = bool(np.any(bias))
    return nblk, (CA1, CB1, CA2, CB2), L, N, NP, has_bias, in_maps


_CACHE = {}


def run(x, W1, b1, W2, b2, alpha, src1, dst1, src2, dst2,
        msg_dt_name="bfloat16", trace=False):
    from concourse import bass_utils
    nblk, Cs, L, N, NP, has_bias, in_maps = _prepare(
        x, W1, b1, W2, b2, alpha, src1, dst1, src2, dst2)
    key = (nblk,) + Cs + (L, has_bias)
    if key not in _CACHE:
        _CACHE[key] = _build(nblk, *Cs, L, has_bias)
    nc = _CACHE[key]
    res = bass_utils.run_bass_kernel_spmd(
        nc, in_maps, core_ids=list(range(NCORES)), trace=trace)
    out = np.concatenate([res.results[k]["out_own"] for k in range(NCORES)],
                         axis=0)[:N]
    return out, res


def kernel(x, W1, b1, W2, b2, alpha, src1, dst1, src2, dst2):
    out, _ = run(np.asarray(x, np.float32), np.asarray(W1, np.float32),
                 np.asarray(b1, np.float32), np.asarray(W2, np.float32),
                 np.asarray(b2, np.float32), np.asarray(alpha, np.float32),
                 np.asarray(src1, np.int32), np.asarray(dst1, np.int32),
                 np.asarray(src2, np.int32), np.asarray(dst2, np.int32))
    return out


# revision 10
# speedup vs baseline: 1.6171x; 1.3476x over previous
"""DelayGNN stage kernel for 8 Trainium2 NeuronCores.

Strategy (graph/data parallel):
  - Nodes sharded across 8 cores (6400 padded nodes each); edge lists
    partitioned by destination core, sorted by (256-node destination block,
    table half, src), padded to uniform chunk counts so one SPMD program
    serves all cores.
  - bf16 message path: the node-feature gather table is bf16 (256B rows),
    scatter one-hots are built on DVE in bf16 (fast perf mode), and both
    scatter and dense matmuls run in bf16 with fp32 PSUM accumulation.
  - Gathers use int16 indices (table split in two halves for the int16
    range) and round-robin across 4 SWDGE queues so descriptor generation
    runs on all four Q7 core pairs concurrently (it is the kernel's
    critical resource). Trailing pad indices are -32768, which the Q7
    ucode trims (no descriptors, no wasted bandwidth).
  - Per layer: hop-1 scatter into per-block PSUM accumulators via one-hot
    matmuls, dense W matmuls, row-layout epilogue (relu + residual + L2
    normalize), then a bf16 AllGather of the new node features; the hop-2
    aggregation (only needed by the next layer) overlaps the AllGather.
"""
import os
import sys
import numpy as np

for _p in ("/opt/trn_rl_repo", "/root/.axon_site/_ro/trn_rl_repo"):
    if os.path.isdir(_p) and _p not in sys.path:
        sys.path.append(_p)

P = 128
BLK = 256
NCORES = 8
# Gather-table split point (both halves must stay under the int16 idx
# range). 32000 makes a typical block's A half exactly 8 chunks = one
# full-size 1024-idx dma_gather call (the per-call fixed cost dominates
# small calls), with B one ~640-idx call.
HALF = int(os.environ.get("KHALF", "31000"))
MAXC = 8          # dma_gather descriptor-ring cap: <=1024 idxs per call
# SWDGE queues (Q7 core pairs) used for gathers
NQ = int(os.environ.get("KNQ", "4"))
# pad idx 0 gathers row 0 (harmless; its one-hot weight is 0). Calls span
# block boundaries, so pads are interior and must be valid indices.
PAD_IDX = int(os.environ.get("KPAD", "0"))
GBUFS = int(os.environ.get("KGBUFS", "6"))


def _wrap_idx(flat):
    """[n] int -> dma_gather idx layout [128, n/16] (wrapped, replicated)."""
    n = len(flat)
    w = np.asarray(flat, np.int16).reshape(n // 16, 16).T  # [16, n/16]
    return np.ascontiguousarray(np.tile(w, (8, 1)))


def _prep_hop(src, dst, norm, n_per_core, nblk):
    """Partition edges by dst core, sort by (dst block, src half, src), pad
    each block to CA + CB chunks of 128. Returns (CA, CB, per-core tables)."""
    core = dst // n_per_core
    percore = []
    cntA = np.zeros(nblk, np.int64)
    cntB = np.zeros(nblk, np.int64)
    for k in range(NCORES):
        sel = core == k
        s, d, w = src[sel], dst[sel] - k * n_per_core, norm[sel]
        blk = d // BLK
        isB = (s >= HALF).astype(np.int64)
        order = np.lexsort((s, isB, blk))
        s, d, w, blk, isB = (a[order] for a in (s, d, w, blk, isB))
        grp = blk * 2 + isB
        cnt = np.bincount(grp, minlength=2 * nblk)
        starts = np.concatenate([[0], np.cumsum(cnt)[:-1]])
        rank = np.arange(len(s)) - starts[grp]
        percore.append((s, d, w, blk, isB, rank))
        cntA = np.maximum(cntA, cnt[0::2])
        cntB = np.maximum(cntB, cnt[1::2])
    # per-block chunk counts (max over cores -> SPMD-uniform program)
    CAb = np.maximum(1, -(-cntA // P)).astype(np.int64)
    CBb = (-(-cntB // P)).astype(np.int64)
    Cgb = CAb + CBb
    choff = np.concatenate([[0], np.cumsum(Cgb)])       # chunk offsets
    aoff = np.concatenate([[0], np.cumsum(CAb)])        # A-chunk offsets
    boff = np.concatenate([[0], np.cumsum(CBb)])        # B-chunk offsets
    J = int(Cgb.sum())
    out = []
    for k in range(NCORES):
        s, d, w, blk, isB, rank = percore[k]
        gix = np.full(J * P, PAD_IDX, np.int64)
        dp = np.zeros(J * P, np.float32)
        wp = np.zeros(J * P, np.float32)
        pos = choff[blk] * P + isB * (CAb[blk] * P) + rank
        gix[pos] = np.where(isB == 1, s - HALF, s)
        dp[pos] = (d % BLK).astype(np.float32)
        wp[pos] = w
        idxA = np.concatenate(
            [_wrap_idx(gix[choff[b] * P:(choff[b] + CAb[b]) * P])
             for b in range(nblk)], axis=1)
        idxB = (np.concatenate(
            [_wrap_idx(gix[(choff[b] + CAb[b]) * P:choff[b + 1] * P])
             for b in range(nblk) if CBb[b]], axis=1)
            if CBb.sum() else None)
        out.append((
            np.ascontiguousarray(dp.reshape(-1, P).T),
            np.ascontiguousarray(wp.reshape(-1, P).T),
            idxA, idxB,
        ))
    return tuple(CAb), tuple(CBb), out


def _edge_norm(src, dst, n):
    ones = np.ones(len(src), np.float32)
    deg_out = np.bincount(src, weights=ones, minlength=n).astype(np.float32)
    deg_in = np.bincount(dst, weights=ones, minlength=n).astype(np.float32)
    inv_out = np.where(deg_out > 0,
                       (1.0 / np.sqrt(np.maximum(deg_out, 1.0))), 0.0)
    inv_in = np.where(deg_in > 0,
                      (1.0 / np.sqrt(np.maximum(deg_in, 1.0))), 0.0)
    return (inv_out[src] * inv_in[dst]).astype(np.float32)


def _softmax(v):
    e = np.exp(v - v.max())
    return (e / e.sum()).astype(np.float32)


def _build(nblk, CA1, CB1, CA2, CB2, L, has_bias):
    """Build the SPMD Bass program. nblk 256-dst blocks per core."""
    import concourse.bass as bass
    import concourse.tile as tile
    from concourse import bacc, mybir
    from concourse.library_config import mlp
    from contextlib import ExitStack

    F32 = mybir.dt.float32
    BF16 = mybir.dt.bfloat16
    I16 = mybir.dt.int16
    NP = nblk * BLK            # nodes per core
    NPAD = NP * NCORES
    HB = min(HALF, NPAD)       # rows in table half A
    NBN = NP // P              # 128-node blocks per core
    CAs, CBs = (np.asarray(CA1), np.asarray(CA2)), (np.asarray(CB1),
                                                     np.asarray(CB2))
    choffs = [np.concatenate([[0], np.cumsum(CAs[h] + CBs[h])])
              for h in range(2)]
    aoffs = [np.concatenate([[0], np.cumsum(CAs[h])]) for h in range(2)]
    boffs = [np.concatenate([[0], np.cumsum(CBs[h])]) for h in range(2)]
    Js = (int(choffs[0][-1]), int(choffs[1][-1]))

    nc = bacc.Bacc("TRN2", target_bir_lowering=False, debug=False,
                   num_devices=NCORES, num_swdge_queues=NQ)

    x_bf = nc.dram_tensor("x_bf", [NPAD, P], BF16, kind="ExternalInput")
    x_own = nc.dram_tensor("x_own", [NP, P], F32, kind="ExternalInput")
    w1d = nc.dram_tensor("w1s", [L, P, P], BF16, kind="ExternalInput")
    w2d = nc.dram_tensor("w2s", [L, P, P], BF16, kind="ExternalInput")
    iotad = nc.dram_tensor("iota", [P, BLK], BF16, kind="ExternalInput")
    dstd = [nc.dram_tensor(f"dst{h}", [P, J], F32, kind="ExternalInput")
            for h, J in ((1, Js[0]), (2, Js[1]))]
    nrmd = [nc.dram_tensor(f"nrm{h}", [P, J], F32, kind="ExternalInput")
            for h, J in ((1, Js[0]), (2, Js[1]))]
    idxad = [nc.dram_tensor(f"idxa{h+1}", [P, int(aoffs[h][-1]) * 8], I16,
                            kind="ExternalInput") for h in range(2)]
    idxbd = [nc.dram_tensor(f"idxb{h+1}", [P, int(boffs[h][-1]) * 8], I16,
                            kind="ExternalInput") if CBs[h].sum() else None
             for h in range(2)]
    if has_bias:
        biasd = nc.dram_tensor("biasb", [L, P, P], F32, kind="ExternalInput")
    out_own = nc.dram_tensor("out_own", [NP, P], F32, kind="ExternalOutput")

    ag_in = [nc.dram_tensor(f"ag_in{t}", [NP, P], BF16, kind="Internal")
             for t in range(L - 1)]
    ag_out = [nc.dram_tensor(f"ag_out{t}", [NPAD, P], BF16, kind="Internal",
                             addr_space="Shared")
              for t in range(L - 1)]

    with tile.TileContext(nc) as tc, ExitStack() as ctx:
        sb = ctx.enter_context(tc.tile_pool(name="sb", bufs=1))
        gq = [ctx.enter_context(tc.tile_pool(name=f"g{q}", bufs=GBUFS))
              for q in range(NQ)]
        ohpool = ctx.enter_context(tc.tile_pool(name="oh", bufs=4))
        accp = ctx.enter_context(
            tc.tile_pool(name="accp", bufs=2, space="PSUM"))
        densep = ctx.enter_context(
            tc.tile_pool(name="densep", bufs=2, space="PSUM"))
        misc = ctx.enter_context(tc.tile_pool(name="misc", bufs=2))

        # --- persistent tiles ---
        t_dst = [sb.tile([P, Js[0]], F32, tag="dst1", name="tdst1"),
                 sb.tile([P, Js[1]], F32, tag="dst2", name="tdst2")]
        t_nrm = [sb.tile([P, Js[0]], F32, tag="nrm1", name="tnrm1"),
                 sb.tile([P, Js[1]], F32, tag="nrm2", name="tnrm2")]
        t_ixa = [sb.tile([P, int(aoffs[h][-1]) * 8], I16, tag=f"ixa{h}",
                         name=f"ixa{h}") for h in range(2)]
        t_ixb = [sb.tile([P, int(boffs[h][-1]) * 8], I16, tag=f"ixb{h}",
                         name=f"ixb{h}") if CBs[h].sum() else None
                 for h in range(2)]
        t_iota = sb.tile([P, BLK], BF16, tag="iota")
        t_w1 = sb.tile([P, L, P], BF16, tag="w1")
        t_w2 = sb.tile([P, L, P], BF16, tag="w2")
        if has_bias:
            t_bias = sb.tile([P, L, P], F32, tag="bias")
        x_rows = [sb.tile([P, NBN, P], F32, tag=f"xr{i}", name=f"xr{i}")
                  for i in range(2)]
        xbf = sb.tile([P, NBN, P], BF16, tag="xbf")
        acc1 = sb.tile([P, nblk, BLK], BF16, tag="acc1")
        acc2 = [sb.tile([P, nblk, BLK], BF16, tag=f"acc2_{i}",
                        name=f"acc2_{i}") for i in range(2)]
        ssum = sb.tile([P, NBN], F32, tag="ssum")
        sinv = sb.tile([P, NBN], F32, tag="sinv")

        nc.gpsimd.load_library(mlp)
        for h in range(2):
            nc.sync.dma_start(t_dst[h][:], dstd[h][:])
            nc.sync.dma_start(t_nrm[h][:], nrmd[h][:])
            nc.sync.dma_start(t_ixa[h][:], idxad[h][:])
            if CBs[h].sum():
                nc.sync.dma_start(t_ixb[h][:], idxbd[h][:])
        nc.sync.dma_start(t_iota[:], iotad[:])
        nc.sync.dma_start(t_w1[:], w1d[:].rearrange("t i o -> i t o"))
        nc.sync.dma_start(t_w2[:], w2d[:].rearrange("t i o -> i t o"))
        if has_bias:
            nc.sync.dma_start(t_bias[:], biasd[:].rearrange("t i o -> i t o"))
        nc.sync.dma_start(x_rows[0][:],
                          x_own[:].rearrange("(a p) f -> p a f", p=P))

        qrr = [0]  # gather queue round-robin state

        def gather_pieces(tab_ap, idx_tile, ch0, C):
            """Gather C chunks (idx-table chunk offset ch0) in pieces of
            <=MAXC chunks. Returns [(first_chunk, piece_tile)]."""
            pieces = []
            for p0 in range(0, C, MAXC):
                pc = min(MAXC, C - p0)
                q = qrr[0] % NQ
                qrr[0] += 1
                ni = pc * P
                xg = gq[q].tile([P, MAXC, P], BF16, tag=f"xg{q}",
                                name=f"xg{q}")
                col0 = (ch0 + p0) * 8
                nc.gpsimd.dma_gather(
                    out_ap=xg[:, 0:pc, :], in_ap=tab_ap,
                    idxs_ap=idx_tile[:, col0:col0 + pc * 8],
                    num_idxs=ni, num_idxs_reg=ni, elem_size=P,
                    queue_num=q)
                pieces.append((p0, xg))
            return pieces

        def scatter_hop(h, acc_sb, tab):
            for b in range(nblk):
                CA, CB = int(CAs[h][b]), int(CBs[h][b])
                Cg = CA + CB
                pa = gather_pieces(tab[0:HB, :], t_ixa[h],
                                   int(aoffs[h][b]), CA)
                pb = (gather_pieces(tab[HB:NPAD, :], t_ixb[h],
                                    int(boffs[h][b]), CB)
                      if CB else [])

                def chunk_ap(c):
                    pieces, cc = (pa, c) if c < CA else (pb, c - CA)
                    for p0, xg in reversed(pieces):
                        if cc >= p0:
                            return xg[:, cc - p0, :]
                    raise AssertionError

                ps = accp.tile([P, BLK], F32, tag="psacc", space="PSUM")
                for c in range(Cg):
                    j = int(choffs[h][b]) + c
                    xsl = chunk_ap(c)
                    oh = ohpool.tile([P, BLK], BF16, tag="oh")
                    nc.vector.tensor_scalar(
                        out=oh[:], in0=t_iota[:],
                        scalar1=t_dst[h][:, j:j + 1],
                        scalar2=t_nrm[h][:, j:j + 1],
                        op0=mybir.AluOpType.is_equal,
                        op1=mybir.AluOpType.mult,
                    )
                    nc.tensor.matmul(out=ps[:], lhsT=xsl, rhs=oh[:],
                                     start=(c == 0), stop=(c == Cg - 1))
                nc.scalar.copy(acc_sb[:, b, :], ps[:])

        for t in range(L):
            tab = x_bf[:] if t == 0 else ag_out[t - 1][:]
            xcur = x_rows[t % 2]
            xnew = x_rows[(t + 1) % 2]
            # hop1 aggregation (this layer)
            scatter_hop(0, acc1, tab)
            # dense + epilogue per 128-node block
            for nb in range(NBN):
                b, hf = nb // 2, nb % 2
                ps = densep.tile([P, P], F32, tag="psd", space="PSUM")
                nc.tensor.matmul(
                    out=ps[:],
                    lhsT=acc1[:, b, hf * P:(hf + 1) * P],
                    rhs=t_w1[:, t, :], start=True, stop=(t == 0))
                if t > 0:
                    nc.tensor.matmul(
                        out=ps[:],
                        lhsT=acc2[(t + 1) % 2][:, b, hf * P:(hf + 1) * P],
                        rhs=t_w2[:, t, :], start=False, stop=True)
                u = misc.tile([P, P], F32, tag="u")
                if has_bias:
                    nc.vector.tensor_tensor(
                        out=u[:], in0=ps[:], in1=t_bias[:, t, :],
                        op=mybir.AluOpType.add)
                    nc.vector.tensor_scalar_max(u[:], u[:], 0.0)
                else:
                    nc.scalar.activation(
                        out=u[:], in_=ps[:],
                        func=mybir.ActivationFunctionType.Relu)
                nc.vector.tensor_tensor(
                    out=xnew[:, nb, :], in0=u[:], in1=xcur[:, nb, :],
                    op=mybir.AluOpType.add)
                sq = misc.tile([P, P], F32, tag="sq")
                nc.scalar.activation(
                    out=sq[:], in_=xnew[:, nb, :],
                    func=mybir.ActivationFunctionType.Square,
                    accum_out=ssum[:, nb:nb + 1])
            nc.scalar.sqrt(sinv[:], ssum[:])
            nc.vector.tensor_scalar_max(sinv[:], sinv[:], 1e-12)
            nc.vector.reciprocal(sinv[:], sinv[:])
            for nb in range(NBN):
                if t < L - 1:
                    nc.vector.tensor_scalar_mul(
                        xbf[:, nb, :], xnew[:, nb, :], sinv[:, nb:nb + 1])
                nc.scalar.activation(
                    out=xnew[:, nb, :], in_=xnew[:, nb, :],
                    func=mybir.ActivationFunctionType.Copy,
                    scale=sinv[:, nb:nb + 1])
            if t < L - 1:
                nc.sync.dma_start(
                    ag_in[t][:].rearrange("(a p) f -> p a f", p=P), xbf[:])
                nc.gpsimd.collective_compute(
                    "AllGather", mybir.AluOpType.bypass,
                    ins=[ag_in[t][:]], outs=[ag_out[t][:]],
                    replica_groups=[list(range(NCORES))],
                )
                # hop2 aggregation for next layer; overlaps the AllGather
                scatter_hop(1, acc2[t % 2], tab)
            else:
                nc.sync.dma_start(
                    out_own[:].rearrange("(a p) f -> p a f", p=P), xnew[:])
    nc.compile()
    return nc


def _prepare(x, W1, b1, W2, b2, alpha, src1, dst1, src2, dst2):
    import ml_dtypes
    BF = ml_dtypes.bfloat16
    N, D = x.shape
    L = W1.shape[0]
    assert D == P
    nblk = -(-N // (NCORES * BLK))
    NP = nblk * BLK
    NPAD = NP * NCORES

    norm1 = _edge_norm(src1, dst1, N)
    norm2 = _edge_norm(src2, dst2, N)
    CA1, CB1, tabs1 = _prep_hop(src1, dst1, norm1, NP, nblk)
    CA2, CB2, tabs2 = _prep_hop(src2, dst2, norm2, NP, nblk)

    a = np.zeros((L, 2), np.float32)
    a[0] = [1.0, 0.0]
    for t in range(1, L):
        a[t] = _softmax(alpha[t].astype(np.float32))
    w1s = (W1 * a[:, 0, None, None]).astype(BF)
    w2s = (W2 * a[:, 1, None, None]).astype(BF)
    bias = (a[:, 0, None] * b1 + a[:, 1, None] * b2).astype(np.float32)
    bias_b = np.broadcast_to(bias[:, None, :], (L, P, P)).copy()

    xpad = np.zeros((NPAD, P), np.float32)
    xpad[:N] = x
    xpad_bf = xpad.astype(BF)
    iota = np.tile(np.arange(BLK, dtype=np.float32), (P, 1)).astype(BF)

    in_maps = []
    for k in range(NCORES):
        m = dict(
            x_bf=xpad_bf, x_own=xpad[k * NP:(k + 1) * NP],
            w1s=w1s, w2s=w2s, iota=iota,
            dst1=tabs1[k][0], nrm1=tabs1[k][1], idxa1=tabs1[k][2],
            dst2=tabs2[k][0], nrm2=tabs2[k][1], idxa2=tabs2[k][2],
        )
        if tabs1[k][3] is not None:
            m["idxb1"] = tabs1[k][3]
        if tabs2[k][3] is not None:
            m["idxb2"] = tabs2[k][3]
        if np.any(bias):
            m["biasb"] = bias_b
        in_maps.append(m)
    has_bias = bool(np.any(bias))
    return nblk, (CA1, CB1, CA2, CB2), L, N, NP, has_bias, in_maps


_CACHE = {}


def run(x, W1, b1, W2, b2, alpha, src1, dst1, src2, dst2,
        msg_dt_name="bfloat16", trace=False):
    from concourse import bass_utils
    nblk, Cs, L, N, NP, has_bias, in_maps = _prepare(
        x, W1, b1, W2, b2, alpha, src1, dst1, src2, dst2)
    key = (nblk,) + Cs + (L, has_bias)
    if key not in _CACHE:
        _CACHE[key] = _build(nblk, *Cs, L, has_bias)
    nc = _CACHE[key]
    res = bass_utils.run_bass_kernel_spmd(
        nc, in_maps, core_ids=list(range(NCORES)), trace=trace)
    out = np.concatenate([res.results[k]["out_own"] for k in range(NCORES)],
                         axis=0)[:N]
    return out, res


def kernel(x, W1, b1, W2, b2, alpha, src1, dst1, src2, dst2):
    out, _ = run(np.asarray(x, np.float32), np.asarray(W1, np.float32),
                 np.asarray(b1, np.float32), np.asarray(W2, np.float32),
                 np.asarray(b2, np.float32), np.asarray(alpha, np.float32),
                 np.asarray(src1, np.int32), np.asarray(dst1, np.int32),
                 np.asarray(src2, np.int32), np.asarray(dst2, np.int32))
    return out


# revision 11
# speedup vs baseline: 1.6774x; 1.0373x over previous
"""DelayGNN stage kernel for 8 Trainium2 NeuronCores.

Strategy (graph/data parallel):
  - Nodes sharded across 8 cores (6400 padded nodes each); edge lists
    partitioned by destination core, sorted by (256-node destination block,
    table half, src), padded to uniform chunk counts so one SPMD program
    serves all cores.
  - bf16 message path: the node-feature gather table is bf16 (256B rows),
    scatter one-hots are built on DVE in bf16 (fast perf mode), and both
    scatter and dense matmuls run in bf16 with fp32 PSUM accumulation.
  - Gathers use int16 indices (table split in two halves for the int16
    range) and round-robin across 4 SWDGE queues so descriptor generation
    runs on all four Q7 core pairs concurrently (it is the kernel's
    critical resource). Trailing pad indices are -32768, which the Q7
    ucode trims (no descriptors, no wasted bandwidth).
  - Per layer: hop-1 scatter into per-block PSUM accumulators via one-hot
    matmuls, dense W matmuls, row-layout epilogue (relu + residual + L2
    normalize), then a bf16 AllGather of the new node features; the hop-2
    aggregation (only needed by the next layer) overlaps the AllGather.
"""
import os
import sys
import numpy as np

for _p in ("/opt/trn_rl_repo", "/root/.axon_site/_ro/trn_rl_repo"):
    if os.path.isdir(_p) and _p not in sys.path:
        sys.path.append(_p)

P = 128
BLK = 256
NCORES = 8
# Gather-table split point (both halves must stay under the int16 idx
# range). 32000 makes a typical block's A half exactly 8 chunks = one
# full-size 1024-idx dma_gather call (the per-call fixed cost dominates
# small calls), with B one ~640-idx call.
HALF = int(os.environ.get("KHALF", "31000"))
MAXC = 8          # dma_gather descriptor-ring cap: <=1024 idxs per call
# SWDGE queues (Q7 core pairs) used for gathers
NQ = int(os.environ.get("KNQ", "4"))
# pad idx 0 gathers row 0 (harmless; its one-hot weight is 0). Calls span
# block boundaries, so pads are interior and must be valid indices.
PAD_IDX = int(os.environ.get("KPAD", "0"))
GBUFS = int(os.environ.get("KGBUFS", "6"))


def _wrap_idx(flat):
    """[n] int -> dma_gather idx layout [128, n/16] (wrapped, replicated)."""
    n = len(flat)
    w = np.asarray(flat, np.int16).reshape(n // 16, 16).T  # [16, n/16]
    return np.ascontiguousarray(np.tile(w, (8, 1)))


def _prep_hop(src, dst, norm, n_per_core, nblk):
    """Partition edges by dst core, sort by (dst block, src half, src), pad
    each block to CA + CB chunks of 128. Returns (CA, CB, per-core tables)."""
    core = dst // n_per_core
    percore = []
    cntA = np.zeros(nblk, np.int64)
    cntB = np.zeros(nblk, np.int64)
    for k in range(NCORES):
        sel = core == k
        s, d, w = src[sel], dst[sel] - k * n_per_core, norm[sel]
        blk = d // BLK
        isB = (s >= HALF).astype(np.int64)
        order = np.lexsort((s, isB, blk))
        s, d, w, blk, isB = (a[order] for a in (s, d, w, blk, isB))
        grp = blk * 2 + isB
        cnt = np.bincount(grp, minlength=2 * nblk)
        starts = np.concatenate([[0], np.cumsum(cnt)[:-1]])
        rank = np.arange(len(s)) - starts[grp]
        percore.append((s, d, w, blk, isB, rank))
        cntA = np.maximum(cntA, cnt[0::2])
        cntB = np.maximum(cntB, cnt[1::2])
    # per-block chunk counts (max over cores -> SPMD-uniform program)
    CAb = np.maximum(1, -(-cntA // P)).astype(np.int64)
    CBb = (-(-cntB // P)).astype(np.int64)
    Cgb = CAb + CBb
    choff = np.concatenate([[0], np.cumsum(Cgb)])       # chunk offsets
    aoff = np.concatenate([[0], np.cumsum(CAb)])        # A-chunk offsets
    boff = np.concatenate([[0], np.cumsum(CBb)])        # B-chunk offsets
    J = int(Cgb.sum())
    out = []
    for k in range(NCORES):
        s, d, w, blk, isB, rank = percore[k]
        gix = np.full(J * P, PAD_IDX, np.int64)
        dp = np.zeros(J * P, np.float32)
        wp = np.zeros(J * P, np.float32)
        pos = choff[blk] * P + isB * (CAb[blk] * P) + rank
        gix[pos] = np.where(isB == 1, s - HALF, s)
        dp[pos] = (d % BLK).astype(np.float32)
        wp[pos] = w
        idxA = np.concatenate(
            [_wrap_idx(gix[choff[b] * P:(choff[b] + CAb[b]) * P])
             for b in range(nblk)], axis=1)
        idxB = (np.concatenate(
            [_wrap_idx(gix[(choff[b] + CAb[b]) * P:choff[b + 1] * P])
             for b in range(nblk) if CBb[b]], axis=1)
            if CBb.sum() else None)
        out.append((
            np.ascontiguousarray(dp.reshape(-1, P).T),
            np.ascontiguousarray(wp.reshape(-1, P).T),
            idxA, idxB,
        ))
    return tuple(CAb), tuple(CBb), out


def _edge_norm(src, dst, n):
    ones = np.ones(len(src), np.float32)
    deg_out = np.bincount(src, weights=ones, minlength=n).astype(np.float32)
    deg_in = np.bincount(dst, weights=ones, minlength=n).astype(np.float32)
    inv_out = np.where(deg_out > 0,
                       (1.0 / np.sqrt(np.maximum(deg_out, 1.0))), 0.0)
    inv_in = np.where(deg_in > 0,
                      (1.0 / np.sqrt(np.maximum(deg_in, 1.0))), 0.0)
    return (inv_out[src] * inv_in[dst]).astype(np.float32)


def _softmax(v):
    e = np.exp(v - v.max())
    return (e / e.sum()).astype(np.float32)


def _build(nblk, CA1, CB1, CA2, CB2, L, has_bias):
    """Build the SPMD Bass program. nblk 256-dst blocks per core."""
    import concourse.bass as bass
    import concourse.tile as tile
    from concourse import bacc, mybir
    from concourse.library_config import mlp
    from contextlib import ExitStack

    F32 = mybir.dt.float32
    BF16 = mybir.dt.bfloat16
    I16 = mybir.dt.int16
    NP = nblk * BLK            # nodes per core
    NPAD = NP * NCORES
    HB = min(HALF, NPAD)       # rows in table half A
    NBN = NP // P              # 128-node blocks per core
    CAs, CBs = (np.asarray(CA1), np.asarray(CA2)), (np.asarray(CB1),
                                                     np.asarray(CB2))
    choffs = [np.concatenate([[0], np.cumsum(CAs[h] + CBs[h])])
              for h in range(2)]
    aoffs = [np.concatenate([[0], np.cumsum(CAs[h])]) for h in range(2)]
    boffs = [np.concatenate([[0], np.cumsum(CBs[h])]) for h in range(2)]
    Js = (int(choffs[0][-1]), int(choffs[1][-1]))

    nc = bacc.Bacc("TRN2", target_bir_lowering=False, debug=False,
                   num_devices=NCORES, num_swdge_queues=NQ)

    x_bf = nc.dram_tensor("x_bf", [NPAD, P], BF16, kind="ExternalInput")
    x_own = nc.dram_tensor("x_own", [NP, P], F32, kind="ExternalInput")
    w1d = nc.dram_tensor("w1s", [L, P, P], BF16, kind="ExternalInput")
    w2d = nc.dram_tensor("w2s", [L, P, P], BF16, kind="ExternalInput")
    iotad = nc.dram_tensor("iota", [P, BLK], BF16, kind="ExternalInput")
    dstd = [nc.dram_tensor(f"dst{h}", [P, J], F32, kind="ExternalInput")
            for h, J in ((1, Js[0]), (2, Js[1]))]
    nrmd = [nc.dram_tensor(f"nrm{h}", [P, J], F32, kind="ExternalInput")
            for h, J in ((1, Js[0]), (2, Js[1]))]
    idxad = [nc.dram_tensor(f"idxa{h+1}", [P, int(aoffs[h][-1]) * 8], I16,
                            kind="ExternalInput") for h in range(2)]
    idxbd = [nc.dram_tensor(f"idxb{h+1}", [P, int(boffs[h][-1]) * 8], I16,
                            kind="ExternalInput") if CBs[h].sum() else None
             for h in range(2)]
    if has_bias:
        biasd = nc.dram_tensor("biasb", [L, P, P], F32, kind="ExternalInput")
    out_own = nc.dram_tensor("out_own", [NP, P], F32, kind="ExternalOutput")

    ag_in = [nc.dram_tensor(f"ag_in{t}", [NP, P], BF16, kind="Internal")
             for t in range(L - 1)]
    ag_out = [nc.dram_tensor(f"ag_out{t}", [NPAD, P], BF16, kind="Internal",
                             addr_space="Shared")
              for t in range(L - 1)]

    with tile.TileContext(nc) as tc, ExitStack() as ctx:
        sb = ctx.enter_context(tc.tile_pool(name="sb", bufs=1))
        gq = [ctx.enter_context(tc.tile_pool(name=f"g{q}", bufs=GBUFS))
              for q in range(NQ)]
        ohpool = ctx.enter_context(tc.tile_pool(name="oh", bufs=4))
        accp = ctx.enter_context(
            tc.tile_pool(name="accp", bufs=2, space="PSUM"))
        densep = ctx.enter_context(
            tc.tile_pool(name="densep", bufs=2, space="PSUM"))
        misc = ctx.enter_context(tc.tile_pool(name="misc", bufs=2))

        # --- persistent tiles ---
        t_dst = [sb.tile([P, Js[0]], F32, tag="dst1", name="tdst1"),
                 sb.tile([P, Js[1]], F32, tag="dst2", name="tdst2")]
        t_nrm = [sb.tile([P, Js[0]], F32, tag="nrm1", name="tnrm1"),
                 sb.tile([P, Js[1]], F32, tag="nrm2", name="tnrm2")]
        t_ixa = [sb.tile([P, int(aoffs[h][-1]) * 8], I16, tag=f"ixa{h}",
                         name=f"ixa{h}") for h in range(2)]
        t_ixb = [sb.tile([P, int(boffs[h][-1]) * 8], I16, tag=f"ixb{h}",
                         name=f"ixb{h}") if CBs[h].sum() else None
                 for h in range(2)]
        t_iota = sb.tile([P, BLK], BF16, tag="iota")
        t_w1 = sb.tile([P, L, P], BF16, tag="w1")
        t_w2 = sb.tile([P, L, P], BF16, tag="w2")
        if has_bias:
            t_bias = sb.tile([P, L, P], F32, tag="bias")
        x_rows = [sb.tile([P, NBN, P], F32, tag=f"xr{i}", name=f"xr{i}")
                  for i in range(2)]
        xbf = sb.tile([P, NBN, P], BF16, tag="xbf")
        acc1 = sb.tile([P, nblk, BLK], BF16, tag="acc1")
        acc2 = [sb.tile([P, nblk, BLK], BF16, tag=f"acc2_{i}",
                        name=f"acc2_{i}") for i in range(2)]
        ssum = sb.tile([P, NBN], F32, tag="ssum")
        sinv = sb.tile([P, NBN], F32, tag="sinv")

        nc.gpsimd.load_library(mlp)
        for h in range(2):
            nc.sync.dma_start(t_dst[h][:], dstd[h][:])
            nc.sync.dma_start(t_nrm[h][:], nrmd[h][:])
            nc.sync.dma_start(t_ixa[h][:], idxad[h][:])
            if CBs[h].sum():
                nc.sync.dma_start(t_ixb[h][:], idxbd[h][:])
        nc.sync.dma_start(t_iota[:], iotad[:])
        nc.sync.dma_start(t_w1[:], w1d[:].rearrange("t i o -> i t o"))
        nc.sync.dma_start(t_w2[:], w2d[:].rearrange("t i o -> i t o"))
        if has_bias:
            nc.sync.dma_start(t_bias[:], biasd[:].rearrange("t i o -> i t o"))
        nc.sync.dma_start(x_rows[0][:],
                          x_own[:].rearrange("(a p) f -> p a f", p=P))

        qrr = [0]  # gather queue round-robin state

        def gather_pieces(tab_ap, idx_tile, ch0, C):
            """Gather C chunks (idx-table chunk offset ch0) in pieces of
            <=MAXC chunks. Returns [(first_chunk, piece_tile)]."""
            pieces = []
            for p0 in range(0, C, MAXC):
                pc = min(MAXC, C - p0)
                q = qrr[0] % NQ
                qrr[0] += 1
                ni = pc * P
                xg = gq[q].tile([P, MAXC, P], BF16, tag=f"xg{q}",
                                name=f"xg{q}")
                col0 = (ch0 + p0) * 8
                nc.gpsimd.dma_gather(
                    out_ap=xg[:, 0:pc, :], in_ap=tab_ap,
                    idxs_ap=idx_tile[:, col0:col0 + pc * 8],
                    num_idxs=ni, num_idxs_reg=ni, elem_size=P,
                    queue_num=q)
                pieces.append((p0, xg))
            return pieces

        def scatter_block(h, b, acc_sb, tab):
            CA, CB = int(CAs[h][b]), int(CBs[h][b])
            Cg = CA + CB
            pa = gather_pieces(tab[0:HB, :], t_ixa[h],
                               int(aoffs[h][b]), CA)
            pb = (gather_pieces(tab[HB:NPAD, :], t_ixb[h],
                                int(boffs[h][b]), CB)
                  if CB else [])

            def chunk_ap(c):
                pieces, cc = (pa, c) if c < CA else (pb, c - CA)
                for p0, xg in reversed(pieces):
                    if cc >= p0:
                        return xg[:, cc - p0, :]
                raise AssertionError

            ps = accp.tile([P, BLK], F32, tag="psacc", space="PSUM")
            for c in range(Cg):
                j = int(choffs[h][b]) + c
                xsl = chunk_ap(c)
                oh = ohpool.tile([P, BLK], BF16, tag="oh")
                nc.vector.tensor_scalar(
                    out=oh[:], in0=t_iota[:],
                    scalar1=t_dst[h][:, j:j + 1],
                    scalar2=t_nrm[h][:, j:j + 1],
                    op0=mybir.AluOpType.is_equal,
                    op1=mybir.AluOpType.mult,
                )
                nc.tensor.matmul(out=ps[:], lhsT=xsl, rhs=oh[:],
                                 start=(c == 0), stop=(c == Cg - 1))
            nc.scalar.copy(acc_sb[:, b, :], ps[:])

        def scatter_hop(h, acc_sb, tab):
            for b in range(nblk):
                scatter_block(h, b, acc_sb, tab)

        for t in range(L):
            tab = x_bf[:] if t == 0 else ag_out[t - 1][:]
            xcur = x_rows[t % 2]
            xnew = x_rows[(t + 1) % 2]
            # hop1 aggregation (this layer)
            scatter_hop(0, acc1, tab)
            # dense + epilogue per 128-node block
            for nb in range(NBN):
                b, hf = nb // 2, nb % 2
                ps = densep.tile([P, P], F32, tag="psd", space="PSUM")
                nc.tensor.matmul(
                    out=ps[:],
                    lhsT=acc1[:, b, hf * P:(hf + 1) * P],
                    rhs=t_w1[:, t, :], start=True, stop=(t == 0))
                if t > 0:
                    nc.tensor.matmul(
                        out=ps[:],
                        lhsT=acc2[(t + 1) % 2][:, b, hf * P:(hf + 1) * P],
                        rhs=t_w2[:, t, :], start=False, stop=True)
                u = misc.tile([P, P], F32, tag="u")
                if has_bias:
                    nc.vector.tensor_tensor(
                        out=u[:], in0=ps[:], in1=t_bias[:, t, :],
                        op=mybir.AluOpType.add)
                    nc.vector.tensor_scalar_max(u[:], u[:], 0.0)
                else:
                    nc.scalar.activation(
                        out=u[:], in_=ps[:],
                        func=mybir.ActivationFunctionType.Relu)
                nc.vector.tensor_tensor(
                    out=xnew[:, nb, :], in0=u[:], in1=xcur[:, nb, :],
                    op=mybir.AluOpType.add)
                sq = misc.tile([P, P], F32, tag="sq")
                nc.scalar.activation(
                    out=sq[:], in_=xnew[:, nb, :],
                    func=mybir.ActivationFunctionType.Square,
                    accum_out=ssum[:, nb:nb + 1])
            nc.scalar.sqrt(sinv[:], ssum[:])
            nc.vector.tensor_scalar_max(sinv[:], sinv[:], 1e-12)
            nc.vector.reciprocal(sinv[:], sinv[:])
            for nb in range(NBN):
                if t < L - 1:
                    nc.vector.tensor_scalar_mul(
                        xbf[:, nb, :], xnew[:, nb, :], sinv[:, nb:nb + 1])
                nc.scalar.activation(
                    out=xnew[:, nb, :], in_=xnew[:, nb, :],
                    func=mybir.ActivationFunctionType.Copy,
                    scale=sinv[:, nb:nb + 1])
            if t < L - 1:
                nc.sync.dma_start(
                    ag_in[t][:].rearrange("(a p) f -> p a f", p=P), xbf[:])
                nc.gpsimd.collective_compute(
                    "AllGather", mybir.AluOpType.bypass,
                    ins=[ag_in[t][:]], outs=[ag_out[t][:]],
                    replica_groups=[list(range(NCORES))],
                )
                # hop2 aggregation for next layer; overlaps the AllGather
                scatter_hop(1, acc2[t % 2], tab)
            else:
                nc.sync.dma_start(
                    out_own[:].rearrange("(a p) f -> p a f", p=P), xnew[:])
    nc.compile()
    return nc


def _prepare(x, W1, b1, W2, b2, alpha, src1, dst1, src2, dst2):
    import ml_dtypes
    BF = ml_dtypes.bfloat16
    N, D = x.shape
    L = W1.shape[0]
    assert D == P
    nblk = -(-N // (NCORES * BLK))
    NP = nblk * BLK
    NPAD = NP * NCORES

    norm1 = _edge_norm(src1, dst1, N)
    norm2 = _edge_norm(src2, dst2, N)
    CA1, CB1, tabs1 = _prep_hop(src1, dst1, norm1, NP, nblk)
    CA2, CB2, tabs2 = _prep_hop(src2, dst2, norm2, NP, nblk)

    a = np.zeros((L, 2), np.float32)
    a[0] = [1.0, 0.0]
    for t in range(1, L):
        a[t] = _softmax(alpha[t].astype(np.float32))
    w1s = (W1 * a[:, 0, None, None]).astype(BF)
    w2s = (W2 * a[:, 1, None, None]).astype(BF)
    bias = (a[:, 0, None] * b1 + a[:, 1, None] * b2).astype(np.float32)
    bias_b = np.broadcast_to(bias[:, None, :], (L, P, P)).copy()

    xpad = np.zeros((NPAD, P), np.float32)
    xpad[:N] = x
    xpad_bf = xpad.astype(BF)
    iota = np.tile(np.arange(BLK, dtype=np.float32), (P, 1)).astype(BF)

    in_maps = []
    for k in range(NCORES):
        m = dict(
            x_bf=xpad_bf, x_own=xpad[k * NP:(k + 1) * NP],
            w1s=w1s, w2s=w2s, iota=iota,
            dst1=tabs1[k][0], nrm1=tabs1[k][1], idxa1=tabs1[k][2],
            dst2=tabs2[k][0], nrm2=tabs2[k][1], idxa2=tabs2[k][2],
        )
        if tabs1[k][3] is not None:
            m["idxb1"] = tabs1[k][3]
        if tabs2[k][3] is not None:
            m["idxb2"] = tabs2[k][3]
        if np.any(bias):
            m["biasb"] = bias_b
        in_maps.append(m)
    has_bias = bool(np.any(bias))
    return nblk, (CA1, CB1, CA2, CB2), L, N, NP, has_bias, in_maps


_CACHE = {}


def run(x, W1, b1, W2, b2, alpha, src1, dst1, src2, dst2,
        msg_dt_name="bfloat16", trace=False):
    from concourse import bass_utils
    nblk, Cs, L, N, NP, has_bias, in_maps = _prepare(
        x, W1, b1, W2, b2, alpha, src1, dst1, src2, dst2)
    key = (nblk,) + Cs + (L, has_bias)
    if key not in _CACHE:
        _CACHE[key] = _build(nblk, *Cs, L, has_bias)
    nc = _CACHE[key]
    res = bass_utils.run_bass_kernel_spmd(
        nc, in_maps, core_ids=list(range(NCORES)), trace=trace)
    out = np.concatenate([res.results[k]["out_own"] for k in range(NCORES)],
                         axis=0)[:N]
    return out, res


def kernel(x, W1, b1, W2, b2, alpha, src1, dst1, src2, dst2):
    out, _ = run(np.asarray(x, np.float32), np.asarray(W1, np.float32),
                 np.asarray(b1, np.float32), np.asarray(W2, np.float32),
                 np.asarray(b2, np.float32), np.asarray(alpha, np.float32),
                 np.asarray(src1, np.int32), np.asarray(dst1, np.int32),
                 np.asarray(src2, np.int32), np.asarray(dst2, np.int32))
    return out


# revision 13
# speedup vs baseline: 1.8500x; 1.1029x over previous
"""DelayGNN stage kernel for 8 Trainium2 NeuronCores.

Strategy (graph/data parallel):
  - Nodes sharded across 8 cores (6400 padded nodes each); edge lists
    partitioned by destination core, sorted by (256-node destination block,
    table half, src), padded to uniform chunk counts so one SPMD program
    serves all cores.
  - bf16 message path: the node-feature gather table is bf16 (256B rows),
    scatter one-hots are built on DVE in bf16 (fast perf mode), and both
    scatter and dense matmuls run in bf16 with fp32 PSUM accumulation.
  - Gathers use int16 indices (table split in two halves for the int16
    range) and round-robin across 4 SWDGE queues so descriptor generation
    runs on all four Q7 core pairs concurrently (it is the kernel's
    critical resource). Trailing pad indices are -32768, which the Q7
    ucode trims (no descriptors, no wasted bandwidth).
  - Per layer: hop-1 scatter into per-block PSUM accumulators via one-hot
    matmuls, dense W matmuls, row-layout epilogue (relu + residual + L2
    normalize), then a bf16 AllGather of the new node features; the hop-2
    aggregation (only needed by the next layer) overlaps the AllGather.
"""
import os
import sys
import numpy as np

for _p in ("/opt/trn_rl_repo", "/root/.axon_site/_ro/trn_rl_repo"):
    if os.path.isdir(_p) and _p not in sys.path:
        sys.path.append(_p)

P = 128
BLK = 256
NCORES = 8
# Gather-table split point (both halves must stay under the int16 idx
# range). 32000 makes a typical block's A half exactly 8 chunks = one
# full-size 1024-idx dma_gather call (the per-call fixed cost dominates
# small calls), with B one ~640-idx call.
HALF = int(os.environ.get("KHALF", "31000"))
MAXC = 8          # dma_gather descriptor-ring cap: <=1024 idxs per call
# SWDGE queues (Q7 core pairs) used for gathers
NQ = int(os.environ.get("KNQ", "4"))
# pad idx 0 gathers row 0 (harmless; its one-hot weight is 0). Calls span
# block boundaries, so pads are interior and must be valid indices.
PAD_IDX = int(os.environ.get("KPAD", "0"))
GBUFS = int(os.environ.get("KGBUFS", "8"))


def _wrap_idx(flat):
    """[n] int -> dma_gather idx layout [128, n/16] (wrapped, replicated)."""
    n = len(flat)
    w = np.asarray(flat, np.int16).reshape(n // 16, 16).T  # [16, n/16]
    return np.ascontiguousarray(np.tile(w, (8, 1)))


def _prep_hop(src, dst, norm, n_per_core, nblk):
    """Partition edges by dst core, sort by (dst block, src half, src), pad
    each block to CA + CB chunks of 128. Returns (CA, CB, per-core tables)."""
    core = dst // n_per_core
    percore = []
    cntA = np.zeros(nblk, np.int64)
    cntB = np.zeros(nblk, np.int64)
    for k in range(NCORES):
        sel = core == k
        s, d, w = src[sel], dst[sel] - k * n_per_core, norm[sel]
        blk = d // BLK
        isB = (s >= HALF).astype(np.int64)
        order = np.lexsort((s, isB, blk))
        s, d, w, blk, isB = (a[order] for a in (s, d, w, blk, isB))
        grp = blk * 2 + isB
        cnt = np.bincount(grp, minlength=2 * nblk)
        starts = np.concatenate([[0], np.cumsum(cnt)[:-1]])
        rank = np.arange(len(s)) - starts[grp]
        percore.append((s, d, w, blk, isB, rank))
        cntA = np.maximum(cntA, cnt[0::2])
        cntB = np.maximum(cntB, cnt[1::2])
    # per-block chunk counts (max over cores -> SPMD-uniform program)
    CAb = np.maximum(1, -(-cntA // P)).astype(np.int64)
    CBb = (-(-cntB // P)).astype(np.int64)
    Cgb = CAb + CBb
    choff = np.concatenate([[0], np.cumsum(Cgb)])       # chunk offsets
    aoff = np.concatenate([[0], np.cumsum(CAb)])        # A-chunk offsets
    boff = np.concatenate([[0], np.cumsum(CBb)])        # B-chunk offsets
    J = int(Cgb.sum())
    out = []
    for k in range(NCORES):
        s, d, w, blk, isB, rank = percore[k]
        gix = np.full(J * P, PAD_IDX, np.int64)
        dp = np.zeros(J * P, np.float32)
        wp = np.zeros(J * P, np.float32)
        pos = choff[blk] * P + isB * (CAb[blk] * P) + rank
        gix[pos] = np.where(isB == 1, s - HALF, s)
        dp[pos] = (d % BLK).astype(np.float32)
        wp[pos] = w
        idxA = np.concatenate(
            [_wrap_idx(gix[choff[b] * P:(choff[b] + CAb[b]) * P])
             for b in range(nblk)], axis=1)
        idxB = (np.concatenate(
            [_wrap_idx(gix[(choff[b] + CAb[b]) * P:choff[b + 1] * P])
             for b in range(nblk) if CBb[b]], axis=1)
            if CBb.sum() else None)
        out.append((
            np.ascontiguousarray(dp.reshape(-1, P).T),
            np.ascontiguousarray(wp.reshape(-1, P).T),
            idxA, idxB,
        ))
    return tuple(CAb), tuple(CBb), out


def _edge_norm(src, dst, n):
    ones = np.ones(len(src), np.float32)
    deg_out = np.bincount(src, weights=ones, minlength=n).astype(np.float32)
    deg_in = np.bincount(dst, weights=ones, minlength=n).astype(np.float32)
    inv_out = np.where(deg_out > 0,
                       (1.0 / np.sqrt(np.maximum(deg_out, 1.0))), 0.0)
    inv_in = np.where(deg_in > 0,
                      (1.0 / np.sqrt(np.maximum(deg_in, 1.0))), 0.0)
    return (inv_out[src] * inv_in[dst]).astype(np.float32)


def _softmax(v):
    e = np.exp(v - v.max())
    return (e / e.sum()).astype(np.float32)


def _build(nblk, CA1, CB1, CA2, CB2, L, has_bias):
    """Build the SPMD Bass program. nblk 256-dst blocks per core."""
    import concourse.bass as bass
    import concourse.tile as tile
    from concourse import bacc, mybir
    from concourse.library_config import mlp
    from contextlib import ExitStack

    F32 = mybir.dt.float32
    BF16 = mybir.dt.bfloat16
    I16 = mybir.dt.int16
    NP = nblk * BLK            # nodes per core
    NPAD = NP * NCORES
    HB = min(HALF, NPAD)       # rows in table half A
    NBN = NP // P              # 128-node blocks per core
    CAs, CBs = (np.asarray(CA1), np.asarray(CA2)), (np.asarray(CB1),
                                                     np.asarray(CB2))
    choffs = [np.concatenate([[0], np.cumsum(CAs[h] + CBs[h])])
              for h in range(2)]
    aoffs = [np.concatenate([[0], np.cumsum(CAs[h])]) for h in range(2)]
    boffs = [np.concatenate([[0], np.cumsum(CBs[h])]) for h in range(2)]
    Js = (int(choffs[0][-1]), int(choffs[1][-1]))

    nc = bacc.Bacc("TRN2", target_bir_lowering=False, debug=False,
                   num_devices=NCORES, num_swdge_queues=NQ)

    x_bf = nc.dram_tensor("x_bf", [NPAD, P], BF16, kind="ExternalInput")
    x_own = nc.dram_tensor("x_own", [NP, P], F32, kind="ExternalInput")
    w1d = nc.dram_tensor("w1s", [L, P, P], BF16, kind="ExternalInput")
    w2d = nc.dram_tensor("w2s", [L, P, P], BF16, kind="ExternalInput")
    iotad = nc.dram_tensor("iota", [P, BLK], BF16, kind="ExternalInput")
    dstd = [nc.dram_tensor(f"dst{h}", [P, J], F32, kind="ExternalInput")
            for h, J in ((1, Js[0]), (2, Js[1]))]
    nrmd = [nc.dram_tensor(f"nrm{h}", [P, J], F32, kind="ExternalInput")
            for h, J in ((1, Js[0]), (2, Js[1]))]
    idxad = [nc.dram_tensor(f"idxa{h+1}", [P, int(aoffs[h][-1]) * 8], I16,
                            kind="ExternalInput") for h in range(2)]
    idxbd = [nc.dram_tensor(f"idxb{h+1}", [P, int(boffs[h][-1]) * 8], I16,
                            kind="ExternalInput") if CBs[h].sum() else None
             for h in range(2)]
    if has_bias:
        biasd = nc.dram_tensor("biasb", [L, P, P], F32, kind="ExternalInput")
    out_own = nc.dram_tensor("out_own", [NP, P], F32, kind="ExternalOutput")

    ag_in = [nc.dram_tensor(f"ag_in{t}", [NP, P], BF16, kind="Internal")
             for t in range(L - 1)]
    ag_out = [nc.dram_tensor(f"ag_out{t}", [NPAD, P], BF16, kind="Internal",
                             addr_space="Shared")
              for t in range(L - 1)]

    with tile.TileContext(nc) as tc, ExitStack() as ctx:
        sb = ctx.enter_context(tc.tile_pool(name="sb", bufs=1))
        gq = [ctx.enter_context(tc.tile_pool(name=f"g{q}", bufs=GBUFS))
              for q in range(NQ)]
        ohpool = ctx.enter_context(tc.tile_pool(name="oh", bufs=4))
        accp = ctx.enter_context(
            tc.tile_pool(name="accp", bufs=2, space="PSUM"))
        densep = ctx.enter_context(
            tc.tile_pool(name="densep", bufs=2, space="PSUM"))
        misc = ctx.enter_context(tc.tile_pool(name="misc", bufs=2))

        # --- persistent tiles ---
        t_dst = [sb.tile([P, Js[0]], F32, tag="dst1", name="tdst1"),
                 sb.tile([P, Js[1]], F32, tag="dst2", name="tdst2")]
        t_nrm = [sb.tile([P, Js[0]], F32, tag="nrm1", name="tnrm1"),
                 sb.tile([P, Js[1]], F32, tag="nrm2", name="tnrm2")]
        t_ixa = [sb.tile([P, int(aoffs[h][-1]) * 8], I16, tag=f"ixa{h}",
                         name=f"ixa{h}") for h in range(2)]
        t_ixb = [sb.tile([P, int(boffs[h][-1]) * 8], I16, tag=f"ixb{h}",
                         name=f"ixb{h}") if CBs[h].sum() else None
                 for h in range(2)]
        t_iota = sb.tile([P, BLK], BF16, tag="iota")
        t_w1 = sb.tile([P, L, P], BF16, tag="w1")
        t_w2 = sb.tile([P, L, P], BF16, tag="w2")
        if has_bias:
            t_bias = sb.tile([P, L, P], F32, tag="bias")
        x_rows = [sb.tile([P, NBN, P], F32, tag=f"xr{i}", name=f"xr{i}")
                  for i in range(2)]
        xbf = sb.tile([P, NBN, P], BF16, tag="xbf")
        acc1 = sb.tile([P, nblk, BLK], BF16, tag="acc1")
        acc2 = [sb.tile([P, nblk, BLK], BF16, tag=f"acc2_{i}",
                        name=f"acc2_{i}") for i in range(2)]
        ssum = sb.tile([P, NBN], F32, tag="ssum")
        sinv = sb.tile([P, NBN], F32, tag="sinv")

        nc.gpsimd.load_library(mlp)
        for h in range(2):
            nc.sync.dma_start(t_dst[h][:], dstd[h][:])
            nc.sync.dma_start(t_nrm[h][:], nrmd[h][:])
            nc.sync.dma_start(t_ixa[h][:], idxad[h][:])
            if CBs[h].sum():
                nc.sync.dma_start(t_ixb[h][:], idxbd[h][:])
        nc.sync.dma_start(t_iota[:], iotad[:])
        nc.sync.dma_start(t_w1[:], w1d[:].rearrange("t i o -> i t o"))
        nc.sync.dma_start(t_w2[:], w2d[:].rearrange("t i o -> i t o"))
        if has_bias:
            nc.sync.dma_start(t_bias[:], biasd[:].rearrange("t i o -> i t o"))
        nc.sync.dma_start(x_rows[0][:],
                          x_own[:].rearrange("(a p) f -> p a f", p=P))

        qrr = [0]  # gather queue round-robin state

        def gather_pieces(tab_ap, idx_tile, ch0, C):
            """Gather C chunks (idx-table chunk offset ch0) in pieces of
            <=MAXC chunks. Returns [(first_chunk, piece_tile)]."""
            pieces = []
            for p0 in range(0, C, MAXC):
                pc = min(MAXC, C - p0)
                q = qrr[0] % NQ
                qrr[0] += 1
                ni = pc * P
                xg = gq[q].tile([P, MAXC, P], BF16, tag=f"xg{q}",
                                name=f"xg{q}")
                col0 = (ch0 + p0) * 8
                nc.gpsimd.dma_gather(
                    out_ap=xg[:, 0:pc, :], in_ap=tab_ap,
                    idxs_ap=idx_tile[:, col0:col0 + pc * 8],
                    num_idxs=ni, num_idxs_reg=ni, elem_size=P,
                    queue_num=q)
                pieces.append((p0, xg))
            return pieces

        def scatter_block(h, b, acc_sb, tab):
            CA, CB = int(CAs[h][b]), int(CBs[h][b])
            Cg = CA + CB
            pa = gather_pieces(tab[0:HB, :], t_ixa[h],
                               int(aoffs[h][b]), CA)
            pb = (gather_pieces(tab[HB:NPAD, :], t_ixb[h],
                                int(boffs[h][b]), CB)
                  if CB else [])

            def chunk_ap(c):
                pieces, cc = (pa, c) if c < CA else (pb, c - CA)
                for p0, xg in reversed(pieces):
                    if cc >= p0:
                        return xg[:, cc - p0, :]
                raise AssertionError

            ps = accp.tile([P, BLK], F32, tag="psacc", space="PSUM")
            for c in range(Cg):
                j = int(choffs[h][b]) + c
                xsl = chunk_ap(c)
                oh = ohpool.tile([P, BLK], BF16, tag="oh")
                nc.vector.tensor_scalar(
                    out=oh[:], in0=t_iota[:],
                    scalar1=t_dst[h][:, j:j + 1],
                    scalar2=t_nrm[h][:, j:j + 1],
                    op0=mybir.AluOpType.is_equal,
                    op1=mybir.AluOpType.mult,
                )
                nc.tensor.matmul(out=ps[:], lhsT=xsl, rhs=oh[:],
                                 start=(c == 0), stop=(c == Cg - 1))
            nc.scalar.copy(acc_sb[:, b, :], ps[:])

        def scatter_hop(h, acc_sb, tab):
            for b in range(nblk):
                scatter_block(h, b, acc_sb, tab)

        for t in range(L):
            tab = x_bf[:] if t == 0 else ag_out[t - 1][:]
            xcur = x_rows[t % 2]
            xnew = x_rows[(t + 1) % 2]
            # hop1 aggregation interleaved with dense + epilogue per block,
            # so only ~one block of epilogue separates the last gather from
            # the AllGather trigger.
            for nb in range(NBN):
                b, hf = nb // 2, nb % 2
                if hf == 0:
                    scatter_block(0, b, acc1, tab)
                ps = densep.tile([P, P], F32, tag="psd", space="PSUM")
                nc.tensor.matmul(
                    out=ps[:],
                    lhsT=acc1[:, b, hf * P:(hf + 1) * P],
                    rhs=t_w1[:, t, :], start=True, stop=(t == 0))
                if t > 0:
                    nc.tensor.matmul(
                        out=ps[:],
                        lhsT=acc2[(t + 1) % 2][:, b, hf * P:(hf + 1) * P],
                        rhs=t_w2[:, t, :], start=False, stop=True)
                u = misc.tile([P, P], F32, tag="u")
                if has_bias:
                    nc.vector.tensor_tensor(
                        out=u[:], in0=ps[:], in1=t_bias[:, t, :],
                        op=mybir.AluOpType.add)
                    nc.vector.tensor_scalar_max(u[:], u[:], 0.0)
                else:
                    nc.scalar.activation(
                        out=u[:], in_=ps[:],
                        func=mybir.ActivationFunctionType.Relu)
                nc.vector.tensor_tensor(
                    out=xnew[:, nb, :], in0=u[:], in1=xcur[:, nb, :],
                    op=mybir.AluOpType.add)
                sq = misc.tile([P, P], F32, tag="sq")
                nc.scalar.activation(
                    out=sq[:], in_=xnew[:, nb, :],
                    func=mybir.ActivationFunctionType.Square,
                    accum_out=ssum[:, nb:nb + 1])
            nc.scalar.sqrt(sinv[:], ssum[:])
            nc.vector.tensor_scalar_max(sinv[:], sinv[:], 1e-12)
            nc.vector.reciprocal(sinv[:], sinv[:])
            for nb in range(NBN):
                if t < L - 1:
                    nc.vector.tensor_scalar_mul(
                        xbf[:, nb, :], xnew[:, nb, :], sinv[:, nb:nb + 1])
                nc.scalar.activation(
                    out=xnew[:, nb, :], in_=xnew[:, nb, :],
                    func=mybir.ActivationFunctionType.Copy,
                    scale=sinv[:, nb:nb + 1])
            if t < L - 1:
                nc.sync.dma_start(
                    ag_in[t][:].rearrange("(a p) f -> p a f", p=P), xbf[:])
                nc.gpsimd.collective_compute(
                    "AllGather", mybir.AluOpType.bypass,
                    ins=[ag_in[t][:]], outs=[ag_out[t][:]],
                    replica_groups=[list(range(NCORES))],
                )
                # hop2 aggregation for next layer; overlaps the AllGather
                scatter_hop(1, acc2[t % 2], tab)
            else:
                nc.sync.dma_start(
                    out_own[:].rearrange("(a p) f -> p a f", p=P), xnew[:])
    nc.compile()
    return nc


def _prepare(x, W1, b1, W2, b2, alpha, src1, dst1, src2, dst2):
    import ml_dtypes
    BF = ml_dtypes.bfloat16
    N, D = x.shape
    L = W1.shape[0]
    assert D == P
    nblk = -(-N // (NCORES * BLK))
    NP = nblk * BLK
    NPAD = NP * NCORES

    norm1 = _edge_norm(src1, dst1, N)
    norm2 = _edge_norm(src2, dst2, N)
    CA1, CB1, tabs1 = _prep_hop(src1, dst1, norm1, NP, nblk)
    CA2, CB2, tabs2 = _prep_hop(src2, dst2, norm2, NP, nblk)

    a = np.zeros((L, 2), np.float32)
    a[0] = [1.0, 0.0]
    for t in range(1, L):
        a[t] = _softmax(alpha[t].astype(np.float32))
    w1s = (W1 * a[:, 0, None, None]).astype(BF)
    w2s = (W2 * a[:, 1, None, None]).astype(BF)
    bias = (a[:, 0, None] * b1 + a[:, 1, None] * b2).astype(np.float32)
    bias_b = np.broadcast_to(bias[:, None, :], (L, P, P)).copy()

    xpad = np.zeros((NPAD, P), np.float32)
    xpad[:N] = x
    xpad_bf = xpad.astype(BF)
    iota = np.tile(np.arange(BLK, dtype=np.float32), (P, 1)).astype(BF)

    in_maps = []
    for k in range(NCORES):
        m = dict(
            x_bf=xpad_bf, x_own=xpad[k * NP:(k + 1) * NP],
            w1s=w1s, w2s=w2s, iota=iota,
            dst1=tabs1[k][0], nrm1=tabs1[k][1], idxa1=tabs1[k][2],
            dst2=tabs2[k][0], nrm2=tabs2[k][1], idxa2=tabs2[k][2],
        )
        if tabs1[k][3] is not None:
            m["idxb1"] = tabs1[k][3]
        if tabs2[k][3] is not None:
            m["idxb2"] = tabs2[k][3]
        if np.any(bias):
            m["biasb"] = bias_b
        in_maps.append(m)
    has_bias = bool(np.any(bias))
    return nblk, (CA1, CB1, CA2, CB2), L, N, NP, has_bias, in_maps


_CACHE = {}


def run(x, W1, b1, W2, b2, alpha, src1, dst1, src2, dst2,
        msg_dt_name="bfloat16", trace=False):
    from concourse import bass_utils
    nblk, Cs, L, N, NP, has_bias, in_maps = _prepare(
        x, W1, b1, W2, b2, alpha, src1, dst1, src2, dst2)
    key = (nblk,) + Cs + (L, has_bias)
    if key not in _CACHE:
        _CACHE[key] = _build(nblk, *Cs, L, has_bias)
    nc = _CACHE[key]
    res = bass_utils.run_bass_kernel_spmd(
        nc, in_maps, core_ids=list(range(NCORES)), trace=trace)
    out = np.concatenate([res.results[k]["out_own"] for k in range(NCORES)],
                         axis=0)[:N]
    return out, res


def kernel(x, W1, b1, W2, b2, alpha, src1, dst1, src2, dst2):
    out, _ = run(np.asarray(x, np.float32), np.asarray(W1, np.float32),
                 np.asarray(b1, np.float32), np.asarray(W2, np.float32),
                 np.asarray(b2, np.float32), np.asarray(alpha, np.float32),
                 np.asarray(src1, np.int32), np.asarray(dst1, np.int32),
                 np.asarray(src2, np.int32), np.asarray(dst2, np.int32))
    return out


# revision 22
# speedup vs baseline: 1.9501x; 1.0541x over previous
"""DelayGNN stage kernel for 8 Trainium2 NeuronCores.

Strategy (graph/data parallel):
  - Nodes sharded across 8 cores (6400 padded nodes each); edge lists
    partitioned by destination core, sorted by (256-node destination block,
    table half, src), padded to uniform chunk counts so one SPMD program
    serves all cores.
  - bf16 message path: the node-feature gather table is bf16 (256B rows),
    scatter one-hots are built on DVE in bf16 (fast perf mode), and both
    scatter and dense matmuls run in bf16 with fp32 PSUM accumulation.
  - Gathers use int16 indices (table split in two halves for the int16
    range) and round-robin across 4 SWDGE queues so descriptor generation
    runs on all four Q7 core pairs concurrently (it is the kernel's
    critical resource). Trailing pad indices are -32768, which the Q7
    ucode trims (no descriptors, no wasted bandwidth).
  - Per layer: hop-1 scatter into per-block PSUM accumulators via one-hot
    matmuls, dense W matmuls, row-layout epilogue (relu + residual + L2
    normalize), then a bf16 AllGather of the new node features; the hop-2
    aggregation (only needed by the next layer) overlaps the AllGather.
"""
import os
import sys
import numpy as np

for _p in ("/opt/trn_rl_repo", "/root/.axon_site/_ro/trn_rl_repo"):
    if os.path.isdir(_p) and _p not in sys.path:
        sys.path.append(_p)

P = 128
BLK = 256
NCORES = 8
# Gather-table split point (both halves must stay under the int16 idx
# range). 32000 makes a typical block's A half exactly 8 chunks = one
# full-size 1024-idx dma_gather call (the per-call fixed cost dominates
# small calls), with B one ~640-idx call.
HALF = int(os.environ.get("KHALF", "31000"))
MAXC = 8          # dma_gather descriptor-ring cap: <=1024 idxs per call
# SWDGE queues (Q7 core pairs) used for gathers
NQ = int(os.environ.get("KNQ", "4"))
# pad idx 0 gathers row 0 (harmless; its one-hot weight is 0). Calls span
# block boundaries, so pads are interior and must be valid indices.
PAD_IDX = int(os.environ.get("KPAD", "0"))
GBUFS = int(os.environ.get("KGBUFS", "8"))


def _wrap_idx(flat):
    """[n] int -> dma_gather idx layout [128, n/16] (wrapped, replicated)."""
    n = len(flat)
    w = np.asarray(flat, np.int16).reshape(n // 16, 16).T  # [16, n/16]
    return np.ascontiguousarray(np.tile(w, (8, 1)))


def _prep_hop(src, dst, norm, n_per_core, nblk):
    """Partition edges by dst core, sort by (dst block, src half, src), pad
    each block to CA + CB chunks of 128. Returns (CA, CB, per-core tables)."""
    core = dst // n_per_core
    percore = []
    cntA = np.zeros(nblk, np.int64)
    cntB = np.zeros(nblk, np.int64)
    for k in range(NCORES):
        sel = core == k
        s, d, w = src[sel], dst[sel] - k * n_per_core, norm[sel]
        blk = d // BLK
        isB = (s >= HALF).astype(np.int64)
        order = np.lexsort((s, isB, blk))
        s, d, w, blk, isB = (a[order] for a in (s, d, w, blk, isB))
        grp = blk * 2 + isB
        cnt = np.bincount(grp, minlength=2 * nblk)
        starts = np.concatenate([[0], np.cumsum(cnt)[:-1]])
        rank = np.arange(len(s)) - starts[grp]
        percore.append((s, d, w, blk, isB, rank))
        cntA = np.maximum(cntA, cnt[0::2])
        cntB = np.maximum(cntB, cnt[1::2])
    # per-block chunk counts (max over cores -> SPMD-uniform program)
    CAb = np.maximum(1, -(-cntA // P)).astype(np.int64)
    CBb = (-(-cntB // P)).astype(np.int64)
    Cgb = CAb + CBb
    choff = np.concatenate([[0], np.cumsum(Cgb)])       # chunk offsets
    aoff = np.concatenate([[0], np.cumsum(CAb)])        # A-chunk offsets
    boff = np.concatenate([[0], np.cumsum(CBb)])        # B-chunk offsets
    J = int(Cgb.sum())
    out = []
    for k in range(NCORES):
        s, d, w, blk, isB, rank = percore[k]
        gix = np.full(J * P, PAD_IDX, np.int64)
        dp = np.zeros(J * P, np.float32)
        wp = np.zeros(J * P, np.float32)
        pos = choff[blk] * P + isB * (CAb[blk] * P) + rank
        gix[pos] = np.where(isB == 1, s - HALF, s)
        dp[pos] = (d % BLK).astype(np.float32)
        wp[pos] = w
        idxA = np.concatenate(
            [_wrap_idx(gix[choff[b] * P:(choff[b] + CAb[b]) * P])
             for b in range(nblk)], axis=1)
        idxB = (np.concatenate(
            [_wrap_idx(gix[(choff[b] + CAb[b]) * P:choff[b + 1] * P])
             for b in range(nblk) if CBb[b]], axis=1)
            if CBb.sum() else None)
        out.append((
            np.ascontiguousarray(dp.reshape(-1, P).T),
            np.ascontiguousarray(wp.reshape(-1, P).T),
            idxA, idxB,
        ))
    return tuple(CAb), tuple(CBb), out


def _edge_norm(src, dst, n):
    ones = np.ones(len(src), np.float32)
    deg_out = np.bincount(src, weights=ones, minlength=n).astype(np.float32)
    deg_in = np.bincount(dst, weights=ones, minlength=n).astype(np.float32)
    inv_out = np.where(deg_out > 0,
                       (1.0 / np.sqrt(np.maximum(deg_out, 1.0))), 0.0)
    inv_in = np.where(deg_in > 0,
                      (1.0 / np.sqrt(np.maximum(deg_in, 1.0))), 0.0)
    return (inv_out[src] * inv_in[dst]).astype(np.float32)


def _softmax(v):
    e = np.exp(v - v.max())
    return (e / e.sum()).astype(np.float32)


def _build(nblk, CA1, CB1, CA2, CB2, L, has_bias, qmap=None):
    """Build the SPMD Bass program. nblk 256-dst blocks per core."""
    import concourse.bass as bass
    import concourse.tile as tile
    from concourse import bacc, mybir
    from concourse.library_config import mlp
    from contextlib import ExitStack

    F32 = mybir.dt.float32
    BF16 = mybir.dt.bfloat16
    I16 = mybir.dt.int16
    NP = nblk * BLK            # nodes per core
    NPAD = NP * NCORES
    HB = min(HALF, NPAD)       # rows in table half A
    NBN = NP // P              # 128-node blocks per core
    CAs, CBs = (np.asarray(CA1), np.asarray(CA2)), (np.asarray(CB1),
                                                     np.asarray(CB2))
    choffs = [np.concatenate([[0], np.cumsum(CAs[h] + CBs[h])])
              for h in range(2)]
    aoffs = [np.concatenate([[0], np.cumsum(CAs[h])]) for h in range(2)]
    boffs = [np.concatenate([[0], np.cumsum(CBs[h])]) for h in range(2)]
    Js = (int(choffs[0][-1]), int(choffs[1][-1]))

    nc = bacc.Bacc("TRN2", target_bir_lowering=False, debug=False,
                   num_devices=NCORES, num_swdge_queues=NQ)

    x_bf = nc.dram_tensor("x_bf", [NPAD, P], BF16, kind="ExternalInput")
    x_own = nc.dram_tensor("x_own", [NP, P], F32, kind="ExternalInput")
    w1d = nc.dram_tensor("w1s", [L, P, P], BF16, kind="ExternalInput")
    w2d = nc.dram_tensor("w2s", [L, P, P], BF16, kind="ExternalInput")
    iotad = nc.dram_tensor("iota", [P, BLK], BF16, kind="ExternalInput")
    dstd = [nc.dram_tensor(f"dst{h}", [P, J], F32, kind="ExternalInput")
            for h, J in ((1, Js[0]), (2, Js[1]))]
    nrmd = [nc.dram_tensor(f"nrm{h}", [P, J], F32, kind="ExternalInput")
            for h, J in ((1, Js[0]), (2, Js[1]))]
    idxad = [nc.dram_tensor(f"idxa{h+1}", [P, int(aoffs[h][-1]) * 8], I16,
                            kind="ExternalInput") for h in range(2)]
    idxbd = [nc.dram_tensor(f"idxb{h+1}", [P, int(boffs[h][-1]) * 8], I16,
                            kind="ExternalInput") if CBs[h].sum() else None
             for h in range(2)]
    if has_bias:
        biasd = nc.dram_tensor("biasb", [L, P, P], F32, kind="ExternalInput")
    out_own = nc.dram_tensor("out_own", [NP, P], F32, kind="ExternalOutput")

    ag_in = [nc.dram_tensor(f"ag_in{t}", [NP, P], BF16, kind="Internal")
             for t in range(L - 1)]
    ag_out = [nc.dram_tensor(f"ag_out{t}", [NPAD, P], BF16, kind="Internal",
                             addr_space="Shared")
              for t in range(L - 1)]

    with tile.TileContext(nc) as tc, ExitStack() as ctx:
        sb = ctx.enter_context(tc.tile_pool(name="sb", bufs=1))
        # One shared gather pool (not per-queue): the buffer rotation must
        # be invariant to queue assignment so the two-pass lane->queue
        # mapping below is a fixed point of the schedule.
        gpool = ctx.enter_context(
            tc.tile_pool(name="gpool", bufs=NQ * GBUFS))
        ohpool = ctx.enter_context(tc.tile_pool(name="oh", bufs=4))
        accp = ctx.enter_context(
            tc.tile_pool(name="accp", bufs=2, space="PSUM"))
        densep = ctx.enter_context(
            tc.tile_pool(name="densep", bufs=2, space="PSUM"))
        misc = ctx.enter_context(tc.tile_pool(name="misc", bufs=2))

        # --- persistent tiles ---
        t_dst = [sb.tile([P, Js[0]], F32, tag="dst1", name="tdst1"),
                 sb.tile([P, Js[1]], F32, tag="dst2", name="tdst2")]
        t_nrm = [sb.tile([P, Js[0]], F32, tag="nrm1", name="tnrm1"),
                 sb.tile([P, Js[1]], F32, tag="nrm2", name="tnrm2")]
        t_ixa = [sb.tile([P, int(aoffs[h][-1]) * 8], I16, tag=f"ixa{h}",
                         name=f"ixa{h}") for h in range(2)]
        t_ixb = [sb.tile([P, int(boffs[h][-1]) * 8], I16, tag=f"ixb{h}",
                         name=f"ixb{h}") if CBs[h].sum() else None
                 for h in range(2)]
        t_iota = sb.tile([P, BLK], BF16, tag="iota")
        t_w1 = sb.tile([P, L, P], BF16, tag="w1")
        t_w2 = sb.tile([P, L, P], BF16, tag="w2")
        if has_bias:
            t_bias = sb.tile([P, L, P], F32, tag="bias")
        x_rows = [sb.tile([P, NBN, P], F32, tag=f"xr{i}", name=f"xr{i}")
                  for i in range(2)]
        xbf = sb.tile([P, NBN, P], BF16, tag="xbf")
        acc1 = sb.tile([P, nblk, BLK], BF16, tag="acc1")
        acc2 = [sb.tile([P, nblk, BLK], BF16, tag=f"acc2_{i}",
                        name=f"acc2_{i}") for i in range(2)]
        ssum = sb.tile([P, NBN], F32, tag="ssum")
        sinv = sb.tile([P, NBN], F32, tag="sinv")

        nc.gpsimd.load_library(mlp)
        for h in range(2):
            nc.sync.dma_start(t_dst[h][:], dstd[h][:])
            nc.sync.dma_start(t_nrm[h][:], nrmd[h][:])
            nc.sync.dma_start(t_ixa[h][:], idxad[h][:])
            if CBs[h].sum():
                nc.sync.dma_start(t_ixb[h][:], idxbd[h][:])
        nc.sync.dma_start(t_iota[:], iotad[:])
        nc.sync.dma_start(t_w1[:], w1d[:].rearrange("t i o -> i t o"))
        nc.sync.dma_start(t_w2[:], w2d[:].rearrange("t i o -> i t o"))
        if has_bias:
            nc.sync.dma_start(t_bias[:], biasd[:].rearrange("t i o -> i t o"))
        nc.sync.dma_start(x_rows[0][:],
                          x_own[:].rearrange("(a p) f -> p a f", p=P))

        qrr = [0]  # gather issue counter
        glist = []  # gather instructions in issue order (for lane->queue map)

        def issue_gather(tab_ap, idx_tile, col0, pc):
            gi = qrr[0]
            qrr[0] += 1
            q = qmap[gi] if qmap is not None else gi % NQ
            ni = pc * P
            xg = gpool.tile([P, MAXC, P], BF16, tag="xg", name="xg")
            inst = nc.gpsimd.dma_gather(
                out_ap=xg[:, 0:pc, :], in_ap=tab_ap,
                idxs_ap=idx_tile[:, col0:col0 + pc * 8],
                num_idxs=ni, num_idxs_reg=ni, elem_size=P,
                queue_num=q)
            glist.append(inst)
            return xg

        def gather_pieces(tab_ap, idx_tile, ch0, C):
            """Gather C chunks (idx-table chunk offset ch0) in pieces of
            <=MAXC chunks. Returns [(first_chunk, piece_tile)]."""
            pieces = []
            for p0 in range(0, C, MAXC):
                pc = min(MAXC, C - p0)
                xg = issue_gather(tab_ap, idx_tile, (ch0 + p0) * 8, pc)
                pieces.append((p0, xg))
            return pieces

        def new_bstream(h, tab):
            """Hop-global gather stream over the B half: full MAXC-chunk
            calls spanning block boundaries (B blocks are only ~5 chunks;
            per-call fixed cost would dominate per-block calls)."""
            return {"h": h, "tab": tab, "pieces": [],
                    "total": int(boffs[h][-1])}

        def bstream_chunk(st, g):
            while g >= len(st["pieces"]) * MAXC:
                p0 = len(st["pieces"]) * MAXC
                pc = min(MAXC, st["total"] - p0)
                xg = issue_gather(st["tab"][HB:NPAD, :], t_ixb[st["h"]],
                                  p0 * 8, pc)
                st["pieces"].append(xg)
            return st["pieces"][g // MAXC][:, g % MAXC, :]

        def scatter_block(h, b, acc_sb, tab, bst):
            CA, CB = int(CAs[h][b]), int(CBs[h][b])
            Cg = CA + CB
            pa = gather_pieces(tab[0:HB, :], t_ixa[h],
                               int(aoffs[h][b]), CA)

            def chunk_ap(c):
                if c >= CA:
                    return bstream_chunk(bst, int(boffs[h][b]) + c - CA)
                for p0, xg in reversed(pa):
                    if c >= p0:
                        return xg[:, c - p0, :]
                raise AssertionError

            ps = accp.tile([P, BLK], F32, tag="psacc", space="PSUM")
            for c in range(Cg):
                j = int(choffs[h][b]) + c
                xsl = chunk_ap(c)
                oh = ohpool.tile([P, BLK], BF16, tag="oh")
                nc.vector.tensor_scalar(
                    out=oh[:], in0=t_iota[:],
                    scalar1=t_dst[h][:, j:j + 1],
                    scalar2=t_nrm[h][:, j:j + 1],
                    op0=mybir.AluOpType.is_equal,
                    op1=mybir.AluOpType.mult,
                )
                nc.tensor.matmul(out=ps[:], lhsT=xsl, rhs=oh[:],
                                 start=(c == 0), stop=(c == Cg - 1))
            nc.scalar.copy(acc_sb[:, b, :], ps[:])

        def scatter_hop(h, acc_sb, tab):
            bst = new_bstream(h, tab)
            for b in range(nblk):
                scatter_block(h, b, acc_sb, tab, bst)

        for t in range(L):
            tab = x_bf[:] if t == 0 else ag_out[t - 1][:]
            xcur = x_rows[t % 2]
            xnew = x_rows[(t + 1) % 2]
            # hop1 aggregation interleaved with dense + epilogue per block,
            # so only ~one block of epilogue separates the last gather from
            # the AllGather trigger.
            bst1 = new_bstream(0, tab)
            for nb in range(NBN):
                b, hf = nb // 2, nb % 2
                if hf == 0:
                    scatter_block(0, b, acc1, tab, bst1)
                ps = densep.tile([P, P], F32, tag="psd", space="PSUM")
                nc.tensor.matmul(
                    out=ps[:],
                    lhsT=acc1[:, b, hf * P:(hf + 1) * P],
                    rhs=t_w1[:, t, :], start=True, stop=(t == 0))
                if t > 0:
                    nc.tensor.matmul(
                        out=ps[:],
                        lhsT=acc2[(t + 1) % 2][:, b, hf * P:(hf + 1) * P],
                        rhs=t_w2[:, t, :], start=False, stop=True)
                u = misc.tile([P, P], F32, tag="u")
                if has_bias:
                    nc.vector.tensor_tensor(
                        out=u[:], in0=ps[:], in1=t_bias[:, t, :],
                        op=mybir.AluOpType.add)
                    nc.vector.tensor_scalar_max(u[:], u[:], 0.0)
                else:
                    nc.scalar.activation(
                        out=u[:], in_=ps[:],
                        func=mybir.ActivationFunctionType.Relu)
                nc.vector.tensor_tensor(
                    out=xnew[:, nb, :], in0=u[:], in1=xcur[:, nb, :],
                    op=mybir.AluOpType.add)
                sq = misc.tile([P, P], F32, tag="sq")
                nc.scalar.activation(
                    out=sq[:], in_=xnew[:, nb, :],
                    func=mybir.ActivationFunctionType.Square,
                    accum_out=ssum[:, nb:nb + 1])
            nc.scalar.sqrt(sinv[:], ssum[:])
            nc.vector.tensor_scalar_max(sinv[:], sinv[:], 1e-12)
            nc.vector.reciprocal(sinv[:], sinv[:])
            for nb in range(NBN):
                if t < L - 1:
                    nc.vector.tensor_scalar_mul(
                        xbf[:, nb, :], xnew[:, nb, :], sinv[:, nb:nb + 1])
                nc.scalar.activation(
                    out=xnew[:, nb, :], in_=xnew[:, nb, :],
                    func=mybir.ActivationFunctionType.Copy,
                    scale=sinv[:, nb:nb + 1])
            if t < L - 1:
                nc.sync.dma_start(
                    ag_in[t][:].rearrange("(a p) f -> p a f", p=P), xbf[:])
                nc.gpsimd.collective_compute(
                    "AllGather", mybir.AluOpType.bypass,
                    ins=[ag_in[t][:]], outs=[ag_out[t][:]],
                    replica_groups=[list(range(NCORES))],
                )
                # hop2 aggregation for next layer; overlaps the AllGather
                scatter_hop(1, acc2[t % 2], tab)
            else:
                nc.sync.dma_start(
                    out_own[:].rearrange("(a p) f -> p a f", p=P), xnew[:])
    nc.compile()
    # Per-gather DMASW lane (issue order), for the two-pass lane->queue map.
    lane_of = {}
    for blk_ in nc.main_func.blocks:
        for i in blk_.instructions:
            if type(i).__name__ == "InstDMAGatherAnt":
                sem = (i.sync_info.on_update[0]
                       if (i.sync_info and i.sync_info.on_update) else None)
                nm = str(sem)
                lane = (int(nm.split("ant_name='DMASW")[1].split("_")[0])
                        if "DMASW" in nm else -1)
                lane_of[i.name] = (lane, i.queue_num)
    lanes = [lane_of.get(inst.ins.name, (-1, 0)) for inst in glist]
    return nc, lanes


def _prepare(x, W1, b1, W2, b2, alpha, src1, dst1, src2, dst2):
    import ml_dtypes
    BF = ml_dtypes.bfloat16
    N, D = x.shape
    L = W1.shape[0]
    assert D == P
    nblk = -(-N // (NCORES * BLK))
    NP = nblk * BLK
    NPAD = NP * NCORES

    norm1 = _edge_norm(src1, dst1, N)
    norm2 = _edge_norm(src2, dst2, N)
    CA1, CB1, tabs1 = _prep_hop(src1, dst1, norm1, NP, nblk)
    CA2, CB2, tabs2 = _prep_hop(src2, dst2, norm2, NP, nblk)

    a = np.zeros((L, 2), np.float32)
    a[0] = [1.0, 0.0]
    for t in range(1, L):
        a[t] = _softmax(alpha[t].astype(np.float32))
    w1s = (W1 * a[:, 0, None, None]).astype(BF)
    w2s = (W2 * a[:, 1, None, None]).astype(BF)
    bias = (a[:, 0, None] * b1 + a[:, 1, None] * b2).astype(np.float32)
    bias_b = np.broadcast_to(bias[:, None, :], (L, P, P)).copy()

    xpad = np.zeros((NPAD, P), np.float32)
    xpad[:N] = x
    xpad_bf = xpad.astype(BF)
    iota = np.tile(np.arange(BLK, dtype=np.float32), (P, 1)).astype(BF)

    in_maps = []
    for k in range(NCORES):
        m = dict(
            x_bf=xpad_bf, x_own=xpad[k * NP:(k + 1) * NP],
            w1s=w1s, w2s=w2s, iota=iota,
            dst1=tabs1[k][0], nrm1=tabs1[k][1], idxa1=tabs1[k][2],
            dst2=tabs2[k][0], nrm2=tabs2[k][1], idxa2=tabs2[k][2],
        )
        if tabs1[k][3] is not None:
            m["idxb1"] = tabs1[k][3]
        if tabs2[k][3] is not None:
            m["idxb2"] = tabs2[k][3]
        if np.any(bias):
            m["biasb"] = bias_b
        in_maps.append(m)
    has_bias = bool(np.any(bias))
    return nblk, (CA1, CB1, CA2, CB2), L, N, NP, has_bias, in_maps


_CACHE = {}


def run(x, W1, b1, W2, b2, alpha, src1, dst1, src2, dst2,
        msg_dt_name="bfloat16", trace=False):
    from concourse import bass_utils
    nblk, Cs, L, N, NP, has_bias, in_maps = _prepare(
        x, W1, b1, W2, b2, alpha, src1, dst1, src2, dst2)
    key = (nblk,) + Cs + (L, has_bias)
    if key not in _CACHE:
        # Pass 1: observe each gather's Tile-assigned DMASW lane; pass 2:
        # pin queue = lane % NQ so every lane serves exactly one SWDGE
        # queue (ucode shadow-sem requirement). The schedule is invariant
        # to queue numbers, so the mapping is a fixed point; verify it.
        _, lanes = _build(nblk, *Cs, L, has_bias)
        qmap = [(ln % NQ if ln >= 0 else 0) for ln, _q in lanes]
        nc, lanes2 = _build(nblk, *Cs, L, has_bias, qmap=qmap)
        mixed = {}
        for ln, q in lanes2:
            if ln >= 0:
                mixed.setdefault(ln, set()).add(q)
        assert all(len(v) == 1 for v in mixed.values()), (
            f"DMASW lane/queue mixing after repin: "
            f"{ {k: sorted(v) for k, v in mixed.items() if len(v) > 1} }"
        )
        _CACHE[key] = nc
    nc = _CACHE[key]
    res = bass_utils.run_bass_kernel_spmd(
        nc, in_maps, core_ids=list(range(NCORES)), trace=trace)
    out = np.concatenate([res.results[k]["out_own"] for k in range(NCORES)],
                         axis=0)[:N]
    return out, res


def kernel(x, W1, b1, W2, b2, alpha, src1, dst1, src2, dst2):
    out, _ = run(np.asarray(x, np.float32), np.asarray(W1, np.float32),
                 np.asarray(b1, np.float32), np.asarray(W2, np.float32),
                 np.asarray(b2, np.float32), np.asarray(alpha, np.float32),
                 np.asarray(src1, np.int32), np.asarray(dst1, np.int32),
                 np.asarray(src2, np.int32), np.asarray(dst2, np.int32))
    return out


# revision 23
# speedup vs baseline: 2.0338x; 1.0429x over previous
"""DelayGNN stage kernel for 8 Trainium2 NeuronCores.

Strategy (graph/data parallel):
  - Nodes sharded across 8 cores (6400 padded nodes each); edge lists
    partitioned by destination core, sorted by (256-node destination block,
    table half, src), padded to uniform chunk counts so one SPMD program
    serves all cores.
  - bf16 message path: the node-feature gather table is bf16 (256B rows),
    scatter one-hots are built on DVE in bf16 (fast perf mode), and both
    scatter and dense matmuls run in bf16 with fp32 PSUM accumulation.
  - Gathers use int16 indices (table split in two halves for the int16
    range) and round-robin across 4 SWDGE queues so descriptor generation
    runs on all four Q7 core pairs concurrently (it is the kernel's
    critical resource). Trailing pad indices are -32768, which the Q7
    ucode trims (no descriptors, no wasted bandwidth).
  - Per layer: hop-1 scatter into per-block PSUM accumulators via one-hot
    matmuls, dense W matmuls, row-layout epilogue (relu + residual + L2
    normalize), then a bf16 AllGather of the new node features; the hop-2
    aggregation (only needed by the next layer) overlaps the AllGather.
"""
import os
import sys
import numpy as np

for _p in ("/opt/trn_rl_repo", "/root/.axon_site/_ro/trn_rl_repo"):
    if os.path.isdir(_p) and _p not in sys.path:
        sys.path.append(_p)

P = 128
BLK = 256
NCORES = 8
# Gather-table split point (both halves must stay under the int16 idx
# range). 32000 makes a typical block's A half exactly 8 chunks = one
# full-size 1024-idx dma_gather call (the per-call fixed cost dominates
# small calls), with B one ~640-idx call.
HALF = int(os.environ.get("KHALF", "31000"))
MAXC = 8          # dma_gather descriptor-ring cap: <=1024 idxs per call
# SWDGE queues (Q7 core pairs) used for gathers
NQ = int(os.environ.get("KNQ", "4"))
# pad idx 0 gathers row 0 (harmless; its one-hot weight is 0). Calls span
# block boundaries, so pads are interior and must be valid indices.
PAD_IDX = int(os.environ.get("KPAD", "0"))
GBUFS = int(os.environ.get("KGBUFS", "9"))


def _wrap_idx(flat):
    """[n] int -> dma_gather idx layout [128, n/16] (wrapped, replicated)."""
    n = len(flat)
    w = np.asarray(flat, np.int16).reshape(n // 16, 16).T  # [16, n/16]
    return np.ascontiguousarray(np.tile(w, (8, 1)))


def _prep_hop(src, dst, norm, n_per_core, nblk):
    """Partition edges by dst core, sort by (dst block, src half, src), pad
    each block to CA + CB chunks of 128. Returns (CA, CB, per-core tables)."""
    core = dst // n_per_core
    percore = []
    cntA = np.zeros(nblk, np.int64)
    cntB = np.zeros(nblk, np.int64)
    for k in range(NCORES):
        sel = core == k
        s, d, w = src[sel], dst[sel] - k * n_per_core, norm[sel]
        blk = d // BLK
        isB = (s >= HALF).astype(np.int64)
        order = np.lexsort((s, isB, blk))
        s, d, w, blk, isB = (a[order] for a in (s, d, w, blk, isB))
        grp = blk * 2 + isB
        cnt = np.bincount(grp, minlength=2 * nblk)
        starts = np.concatenate([[0], np.cumsum(cnt)[:-1]])
        rank = np.arange(len(s)) - starts[grp]
        percore.append((s, d, w, blk, isB, rank))
        cntA = np.maximum(cntA, cnt[0::2])
        cntB = np.maximum(cntB, cnt[1::2])
    # per-block chunk counts (max over cores -> SPMD-uniform program)
    CAb = np.maximum(1, -(-cntA // P)).astype(np.int64)
    CBb = (-(-cntB // P)).astype(np.int64)
    Cgb = CAb + CBb
    choff = np.concatenate([[0], np.cumsum(Cgb)])       # chunk offsets
    aoff = np.concatenate([[0], np.cumsum(CAb)])        # A-chunk offsets
    boff = np.concatenate([[0], np.cumsum(CBb)])        # B-chunk offsets
    J = int(Cgb.sum())
    out = []
    for k in range(NCORES):
        s, d, w, blk, isB, rank = percore[k]
        gix = np.full(J * P, PAD_IDX, np.int64)
        dp = np.zeros(J * P, np.float32)
        wp = np.zeros(J * P, np.float32)
        pos = choff[blk] * P + isB * (CAb[blk] * P) + rank
        gix[pos] = np.where(isB == 1, s - HALF, s)
        dp[pos] = (d % BLK).astype(np.float32)
        wp[pos] = w
        idxA = np.concatenate(
            [_wrap_idx(gix[choff[b] * P:(choff[b] + CAb[b]) * P])
             for b in range(nblk)], axis=1)
        idxB = (np.concatenate(
            [_wrap_idx(gix[(choff[b] + CAb[b]) * P:choff[b + 1] * P])
             for b in range(nblk) if CBb[b]], axis=1)
            if CBb.sum() else None)
        out.append((
            np.ascontiguousarray(dp.reshape(-1, P).T),
            np.ascontiguousarray(wp.reshape(-1, P).T),
            idxA, idxB,
        ))
    return tuple(CAb), tuple(CBb), out


def _edge_norm(src, dst, n):
    ones = np.ones(len(src), np.float32)
    deg_out = np.bincount(src, weights=ones, minlength=n).astype(np.float32)
    deg_in = np.bincount(dst, weights=ones, minlength=n).astype(np.float32)
    inv_out = np.where(deg_out > 0,
                       (1.0 / np.sqrt(np.maximum(deg_out, 1.0))), 0.0)
    inv_in = np.where(deg_in > 0,
                      (1.0 / np.sqrt(np.maximum(deg_in, 1.0))), 0.0)
    return (inv_out[src] * inv_in[dst]).astype(np.float32)


def _softmax(v):
    e = np.exp(v - v.max())
    return (e / e.sum()).astype(np.float32)


def _build(nblk, CA1, CB1, CA2, CB2, L, has_bias, qmap=None):
    """Build the SPMD Bass program. nblk 256-dst blocks per core."""
    import concourse.bass as bass
    import concourse.tile as tile
    from concourse import bacc, mybir
    from concourse.library_config import mlp
    from contextlib import ExitStack

    F32 = mybir.dt.float32
    BF16 = mybir.dt.bfloat16
    I16 = mybir.dt.int16
    NP = nblk * BLK            # nodes per core
    NPAD = NP * NCORES
    HB = min(HALF, NPAD)       # rows in table half A
    NBN = NP // P              # 128-node blocks per core
    CAs, CBs = (np.asarray(CA1), np.asarray(CA2)), (np.asarray(CB1),
                                                     np.asarray(CB2))
    choffs = [np.concatenate([[0], np.cumsum(CAs[h] + CBs[h])])
              for h in range(2)]
    aoffs = [np.concatenate([[0], np.cumsum(CAs[h])]) for h in range(2)]
    boffs = [np.concatenate([[0], np.cumsum(CBs[h])]) for h in range(2)]
    Js = (int(choffs[0][-1]), int(choffs[1][-1]))

    nc = bacc.Bacc("TRN2", target_bir_lowering=False, debug=False,
                   num_devices=NCORES, num_swdge_queues=NQ)

    x_bf = nc.dram_tensor("x_bf", [NPAD, P], BF16, kind="ExternalInput")
    x_own = nc.dram_tensor("x_own", [NP, P], F32, kind="ExternalInput")
    w1d = nc.dram_tensor("w1s", [L, P, P], BF16, kind="ExternalInput")
    w2d = nc.dram_tensor("w2s", [L, P, P], BF16, kind="ExternalInput")
    iotad = nc.dram_tensor("iota", [P, BLK], BF16, kind="ExternalInput")
    dstd = [nc.dram_tensor(f"dst{h}", [P, J], F32, kind="ExternalInput")
            for h, J in ((1, Js[0]), (2, Js[1]))]
    nrmd = [nc.dram_tensor(f"nrm{h}", [P, J], F32, kind="ExternalInput")
            for h, J in ((1, Js[0]), (2, Js[1]))]
    idxad = [nc.dram_tensor(f"idxa{h+1}", [P, int(aoffs[h][-1]) * 8], I16,
                            kind="ExternalInput") for h in range(2)]
    idxbd = [nc.dram_tensor(f"idxb{h+1}", [P, int(boffs[h][-1]) * 8], I16,
                            kind="ExternalInput") if CBs[h].sum() else None
             for h in range(2)]
    if has_bias:
        biasd = nc.dram_tensor("biasb", [L, P, P], F32, kind="ExternalInput")
    out_own = nc.dram_tensor("out_own", [NP, P], F32, kind="ExternalOutput")

    ag_in = [nc.dram_tensor(f"ag_in{t}", [NP, P], BF16, kind="Internal")
             for t in range(L - 1)]
    ag_out = [nc.dram_tensor(f"ag_out{t}", [NPAD, P], BF16, kind="Internal",
                             addr_space="Shared")
              for t in range(L - 1)]

    with tile.TileContext(nc) as tc, ExitStack() as ctx:
        sb = ctx.enter_context(tc.tile_pool(name="sb", bufs=1))
        # One shared gather pool (not per-queue): the buffer rotation must
        # be invariant to queue assignment so the two-pass lane->queue
        # mapping below is a fixed point of the schedule.
        gpool = ctx.enter_context(
            tc.tile_pool(name="gpool", bufs=NQ * GBUFS))
        ohpool = ctx.enter_context(tc.tile_pool(name="oh", bufs=6))
        accp = ctx.enter_context(
            tc.tile_pool(name="accp", bufs=2, space="PSUM"))
        densep = ctx.enter_context(
            tc.tile_pool(name="densep", bufs=2, space="PSUM"))
        misc = ctx.enter_context(tc.tile_pool(name="misc", bufs=4))

        # --- persistent tiles ---
        t_dst = [sb.tile([P, Js[0]], F32, tag="dst1", name="tdst1"),
                 sb.tile([P, Js[1]], F32, tag="dst2", name="tdst2")]
        t_nrm = [sb.tile([P, Js[0]], F32, tag="nrm1", name="tnrm1"),
                 sb.tile([P, Js[1]], F32, tag="nrm2", name="tnrm2")]
        t_ixa = [sb.tile([P, int(aoffs[h][-1]) * 8], I16, tag=f"ixa{h}",
                         name=f"ixa{h}") for h in range(2)]
        t_ixb = [sb.tile([P, int(boffs[h][-1]) * 8], I16, tag=f"ixb{h}",
                         name=f"ixb{h}") if CBs[h].sum() else None
                 for h in range(2)]
        t_iota = sb.tile([P, BLK], BF16, tag="iota")
        t_w1 = sb.tile([P, L, P], BF16, tag="w1")
        t_w2 = sb.tile([P, L, P], BF16, tag="w2")
        if has_bias:
            t_bias = sb.tile([P, L, P], F32, tag="bias")
        x_rows = [sb.tile([P, NBN, P], F32, tag=f"xr{i}", name=f"xr{i}")
                  for i in range(2)]
        xbf = sb.tile([P, NBN, P], BF16, tag="xbf")
        acc1 = sb.tile([P, nblk, BLK], BF16, tag="acc1")
        acc2 = sb.tile([P, nblk, BLK], BF16, tag="acc2")
        ssum = sb.tile([P, NBN], F32, tag="ssum")
        sinv = sb.tile([P, NBN], F32, tag="sinv")

        nc.gpsimd.load_library(mlp)
        for h in range(2):
            nc.sync.dma_start(t_dst[h][:], dstd[h][:])
            nc.sync.dma_start(t_nrm[h][:], nrmd[h][:])
            nc.sync.dma_start(t_ixa[h][:], idxad[h][:])
            if CBs[h].sum():
                nc.sync.dma_start(t_ixb[h][:], idxbd[h][:])
        nc.sync.dma_start(t_iota[:], iotad[:])
        nc.sync.dma_start(t_w1[:], w1d[:].rearrange("t i o -> i t o"))
        nc.sync.dma_start(t_w2[:], w2d[:].rearrange("t i o -> i t o"))
        if has_bias:
            nc.sync.dma_start(t_bias[:], biasd[:].rearrange("t i o -> i t o"))
        nc.sync.dma_start(x_rows[0][:],
                          x_own[:].rearrange("(a p) f -> p a f", p=P))

        qrr = [0]  # gather issue counter
        glist = []  # gather instructions in issue order (for lane->queue map)

        def issue_gather(tab_ap, idx_tile, col0, pc):
            gi = qrr[0]
            qrr[0] += 1
            q = qmap[gi] if qmap is not None else gi % NQ
            ni = pc * P
            xg = gpool.tile([P, MAXC, P], BF16, tag="xg", name="xg")
            inst = nc.gpsimd.dma_gather(
                out_ap=xg[:, 0:pc, :], in_ap=tab_ap,
                idxs_ap=idx_tile[:, col0:col0 + pc * 8],
                num_idxs=ni, num_idxs_reg=ni, elem_size=P,
                queue_num=q)
            glist.append(inst)
            return xg

        def gather_pieces(tab_ap, idx_tile, ch0, C):
            """Gather C chunks (idx-table chunk offset ch0) in pieces of
            <=MAXC chunks. Returns [(first_chunk, piece_tile)]."""
            pieces = []
            for p0 in range(0, C, MAXC):
                pc = min(MAXC, C - p0)
                xg = issue_gather(tab_ap, idx_tile, (ch0 + p0) * 8, pc)
                pieces.append((p0, xg))
            return pieces

        def new_bstream(h, tab):
            """Hop-global gather stream over the B half: full MAXC-chunk
            calls spanning block boundaries (B blocks are only ~5 chunks;
            per-call fixed cost would dominate per-block calls)."""
            return {"h": h, "tab": tab, "pieces": [],
                    "total": int(boffs[h][-1])}

        def bstream_chunk(st, g):
            while g >= len(st["pieces"]) * MAXC:
                p0 = len(st["pieces"]) * MAXC
                pc = min(MAXC, st["total"] - p0)
                xg = issue_gather(st["tab"][HB:NPAD, :], t_ixb[st["h"]],
                                  p0 * 8, pc)
                st["pieces"].append(xg)
            return st["pieces"][g // MAXC][:, g % MAXC, :]

        def scatter_block(h, b, acc_sb, tab, bst):
            CA, CB = int(CAs[h][b]), int(CBs[h][b])
            Cg = CA + CB
            pa = gather_pieces(tab[0:HB, :], t_ixa[h],
                               int(aoffs[h][b]), CA)

            def chunk_ap(c):
                if c >= CA:
                    return bstream_chunk(bst, int(boffs[h][b]) + c - CA)
                for p0, xg in reversed(pa):
                    if c >= p0:
                        return xg[:, c - p0, :]
                raise AssertionError

            ps = accp.tile([P, BLK], F32, tag="psacc", space="PSUM")
            for c in range(Cg):
                j = int(choffs[h][b]) + c
                xsl = chunk_ap(c)
                oh = ohpool.tile([P, BLK], BF16, tag="oh")
                nc.vector.tensor_scalar(
                    out=oh[:], in0=t_iota[:],
                    scalar1=t_dst[h][:, j:j + 1],
                    scalar2=t_nrm[h][:, j:j + 1],
                    op0=mybir.AluOpType.is_equal,
                    op1=mybir.AluOpType.mult,
                )
                nc.tensor.matmul(out=ps[:], lhsT=xsl, rhs=oh[:],
                                 start=(c == 0), stop=(c == Cg - 1))
            nc.scalar.copy(acc_sb[:, b, :], ps[:])

        def scatter_hop(h, acc_sb, tab):
            bst = new_bstream(h, tab)
            for b in range(nblk):
                scatter_block(h, b, acc_sb, tab, bst)

        for t in range(L):
            tab = x_bf[:] if t == 0 else ag_out[t - 1][:]
            xcur = x_rows[t % 2]
            xnew = x_rows[(t + 1) % 2]
            # hop1 aggregation interleaved with dense + epilogue per block,
            # so only ~one block of epilogue separates the last gather from
            # the AllGather trigger.
            bst1 = new_bstream(0, tab)
            for nb in range(NBN):
                b, hf = nb // 2, nb % 2
                if hf == 0:
                    scatter_block(0, b, acc1, tab, bst1)
                ps = densep.tile([P, P], F32, tag="psd", space="PSUM")
                nc.tensor.matmul(
                    out=ps[:],
                    lhsT=acc1[:, b, hf * P:(hf + 1) * P],
                    rhs=t_w1[:, t, :], start=True, stop=(t == 0))
                if t > 0:
                    nc.tensor.matmul(
                        out=ps[:],
                        lhsT=acc2[:, b, hf * P:(hf + 1) * P],
                        rhs=t_w2[:, t, :], start=False, stop=True)
                u = misc.tile([P, P], F32, tag="u")
                if has_bias:
                    nc.vector.tensor_tensor(
                        out=u[:], in0=ps[:], in1=t_bias[:, t, :],
                        op=mybir.AluOpType.add)
                    nc.vector.tensor_scalar_max(u[:], u[:], 0.0)
                else:
                    nc.scalar.activation(
                        out=u[:], in_=ps[:],
                        func=mybir.ActivationFunctionType.Relu)
                nc.vector.tensor_tensor(
                    out=xnew[:, nb, :], in0=u[:], in1=xcur[:, nb, :],
                    op=mybir.AluOpType.add)
                sq = misc.tile([P, P], F32, tag="sq")
                nc.scalar.activation(
                    out=sq[:], in_=xnew[:, nb, :],
                    func=mybir.ActivationFunctionType.Square,
                    accum_out=ssum[:, nb:nb + 1])
            nc.scalar.sqrt(sinv[:], ssum[:])
            nc.vector.tensor_scalar_max(sinv[:], sinv[:], 1e-12)
            nc.vector.reciprocal(sinv[:], sinv[:])
            for nb in range(NBN):
                if t < L - 1:
                    nc.vector.tensor_scalar_mul(
                        xbf[:, nb, :], xnew[:, nb, :], sinv[:, nb:nb + 1])
                nc.scalar.activation(
                    out=xnew[:, nb, :], in_=xnew[:, nb, :],
                    func=mybir.ActivationFunctionType.Copy,
                    scale=sinv[:, nb:nb + 1])
            if t < L - 1:
                nc.sync.dma_start(
                    ag_in[t][:].rearrange("(a p) f -> p a f", p=P), xbf[:])
                nc.gpsimd.collective_compute(
                    "AllGather", mybir.AluOpType.bypass,
                    ins=[ag_in[t][:]], outs=[ag_out[t][:]],
                    replica_groups=[list(range(NCORES))],
                )
                # hop2 aggregation for next layer; overlaps the AllGather
                scatter_hop(1, acc2, tab)
            else:
                nc.sync.dma_start(
                    out_own[:].rearrange("(a p) f -> p a f", p=P), xnew[:])
    nc.compile()
    # Per-gather DMASW lane (issue order), for the two-pass lane->queue map.
    lane_of = {}
    for blk_ in nc.main_func.blocks:
        for i in blk_.instructions:
            if type(i).__name__ == "InstDMAGatherAnt":
                sem = (i.sync_info.on_update[0]
                       if (i.sync_info and i.sync_info.on_update) else None)
                nm = str(sem)
                lane = (int(nm.split("ant_name='DMASW")[1].split("_")[0])
                        if "DMASW" in nm else -1)
                lane_of[i.name] = (lane, i.queue_num)
    lanes = [lane_of.get(inst.ins.name, (-1, 0)) for inst in glist]
    return nc, lanes


def _prepare(x, W1, b1, W2, b2, alpha, src1, dst1, src2, dst2):
    import ml_dtypes
    BF = ml_dtypes.bfloat16
    N, D = x.shape
    L = W1.shape[0]
    assert D == P
    nblk = -(-N // (NCORES * BLK))
    NP = nblk * BLK
    NPAD = NP * NCORES

    norm1 = _edge_norm(src1, dst1, N)
    norm2 = _edge_norm(src2, dst2, N)
    CA1, CB1, tabs1 = _prep_hop(src1, dst1, norm1, NP, nblk)
    CA2, CB2, tabs2 = _prep_hop(src2, dst2, norm2, NP, nblk)

    a = np.zeros((L, 2), np.float32)
    a[0] = [1.0, 0.0]
    for t in range(1, L):
        a[t] = _softmax(alpha[t].astype(np.float32))
    w1s = (W1 * a[:, 0, None, None]).astype(BF)
    w2s = (W2 * a[:, 1, None, None]).astype(BF)
    bias = (a[:, 0, None] * b1 + a[:, 1, None] * b2).astype(np.float32)
    bias_b = np.broadcast_to(bias[:, None, :], (L, P, P)).copy()

    xpad = np.zeros((NPAD, P), np.float32)
    xpad[:N] = x
    xpad_bf = xpad.astype(BF)
    iota = np.tile(np.arange(BLK, dtype=np.float32), (P, 1)).astype(BF)

    in_maps = []
    for k in range(NCORES):
        m = dict(
            x_bf=xpad_bf, x_own=xpad[k * NP:(k + 1) * NP],
            w1s=w1s, w2s=w2s, iota=iota,
            dst1=tabs1[k][0], nrm1=tabs1[k][1], idxa1=tabs1[k][2],
            dst2=tabs2[k][0], nrm2=tabs2[k][1], idxa2=tabs2[k][2],
        )
        if tabs1[k][3] is not None:
            m["idxb1"] = tabs1[k][3]
        if tabs2[k][3] is not None:
            m["idxb2"] = tabs2[k][3]
        if np.any(bias):
            m["biasb"] = bias_b
        in_maps.append(m)
    has_bias = bool(np.any(bias))
    return nblk, (CA1, CB1, CA2, CB2), L, N, NP, has_bias, in_maps


_CACHE = {}


def run(x, W1, b1, W2, b2, alpha, src1, dst1, src2, dst2,
        msg_dt_name="bfloat16", trace=False):
    from concourse import bass_utils
    nblk, Cs, L, N, NP, has_bias, in_maps = _prepare(
        x, W1, b1, W2, b2, alpha, src1, dst1, src2, dst2)
    key = (nblk,) + Cs + (L, has_bias)
    if key not in _CACHE:
        # Pass 1: observe each gather's Tile-assigned DMASW lane; pass 2:
        # pin queue = lane % NQ so every lane serves exactly one SWDGE
        # queue (ucode shadow-sem requirement). The schedule is invariant
        # to queue numbers, so the mapping is a fixed point; verify it.
        _, lanes = _build(nblk, *Cs, L, has_bias)
        qmap = [(ln % NQ if ln >= 0 else 0) for ln, _q in lanes]
        nc, lanes2 = _build(nblk, *Cs, L, has_bias, qmap=qmap)
        mixed = {}
        for ln, q in lanes2:
            if ln >= 0:
                mixed.setdefault(ln, set()).add(q)
        assert all(len(v) == 1 for v in mixed.values()), (
            f"DMASW lane/queue mixing after repin: "
            f"{ {k: sorted(v) for k, v in mixed.items() if len(v) > 1} }"
        )
        _CACHE[key] = nc
    nc = _CACHE[key]
    res = bass_utils.run_bass_kernel_spmd(
        nc, in_maps, core_ids=list(range(NCORES)), trace=trace)
    out = np.concatenate([res.results[k]["out_own"] for k in range(NCORES)],
                         axis=0)[:N]
    return out, res


def kernel(x, W1, b1, W2, b2, alpha, src1, dst1, src2, dst2):
    out, _ = run(np.asarray(x, np.float32), np.asarray(W1, np.float32),
                 np.asarray(b1, np.float32), np.asarray(W2, np.float32),
                 np.asarray(b2, np.float32), np.asarray(alpha, np.float32),
                 np.asarray(src1, np.int32), np.asarray(dst1, np.int32),
                 np.asarray(src2, np.int32), np.asarray(dst2, np.int32))
    return out
